# revision 20
# baseline (speedup 1.0000x reference)
"""Trainium2 Bass kernel for nn_DifferentiableBiquadChain.

16 cascaded biquads over (16, 262144) audio. The whole cascade is one LTI
system with a 32-dim state; we decompose each batch row's sequence into 2048
chunks of 128 samples and compute:

  phase 1a: per-chunk end-state contribution  D_c = U @ x_c        (TensorE)
  phase 2:  boundary-state scan S_{c+1} = A^128 S_c + D_c, done as a
            radix-4 hierarchical matmul scan over precomputed A-powers
  phase 1b: per-chunk zero-state response  y_zs = Toeplitz(g) @ x_c (TensorE)
  phase 3:  correction y_c = y_zs + Kmat @ S_c, PSUM-accumulated

All matrices (impulse response g, U, Kmat, scan-level A-power blocks) are
precomputed on host in float64 from `params`; the device does only fp32
matmuls + copies. Data-parallel: 2 batch rows per core on 8 cores.
"""
import math

import numpy as np

FS = 96000.0
N_BIQUADS = 16
HPF_FREQ_RANGE = (20.0, 500.0)
LPF_FREQ_RANGE = (5000.0, 20000.0)
SHELF_FREQ_RANGE = (50.0, 16000.0)
PEAK_FREQ_RANGE = (100.0, 15000.0)
Q_RANGE = (0.5, 16.0)
GAIN_RANGE = (-24.0, 24.0)
BROADBAND_RANGE = (-60.0, 0.0)

T = 262144
L = 512          # chunk length
NQ = L // 128    # K-quarters per chunk
C = T // L       # 512 chunks per row
NBLK = C // 128  # 4 chunk-blocks per row
NST = 2 * N_BIQUADS   # 32 state dims
ROWS_PER_CORE = 2
N_CORES = 8
SCAN_SIZES = [512, 128, 32, 8]  # radix-4 upsweep input sizes
N_LEV = len(SCAN_SIZES)


# ---------------------------------------------------------------------------
# host-side math (float64)
# ---------------------------------------------------------------------------

def _denorm_log(norm, lo, hi):
    return np.exp(math.log(lo) + norm * (math.log(hi) - math.log(lo)))


def _coef_highpass(fc, Q):
    w0 = 2.0 * math.pi * fc / FS
    alpha = np.sin(w0) / (2.0 * Q)
    c = np.cos(w0)
    b0 = (1 + c) / 2; b1 = -(1 + c); b2 = (1 + c) / 2
    a0 = 1 + alpha; a1 = -2 * c; a2 = 1 - alpha
    return b0 / a0, b1 / a0, b2 / a0, a1 / a0, a2 / a0


def _coef_lowpass(fc, Q):
    w0 = 2.0 * math.pi * fc / FS
    alpha = np.sin(w0) / (2.0 * Q)
    c = np.cos(w0)
    b0 = (1 - c) / 2; b1 = 1 - c; b2 = (1 - c) / 2
    a0 = 1 + alpha; a1 = -2 * c; a2 = 1 - alpha
    return b0 / a0, b1 / a0, b2 / a0, a1 / a0, a2 / a0


def _coef_lowshelf(fc, gain_db, Q):
    A = 10.0 ** (gain_db / 40.0)
    w0 = 2.0 * math.pi * fc / FS
    alpha = np.sin(w0) / (2.0 * Q)
    c = np.cos(w0)
    sA = np.sqrt(A)
    b0 = A * (A + 1 - (A - 1) * c + 2 * sA * alpha)
    b1 = 2 * A * (A - 1 - (A + 1) * c)
    b2 = A * (A + 1 - (A - 1) * c - 2 * sA * alpha)
    a0 = A + 1 + (A - 1) * c + 2 * sA * alpha
    a1 = -2 * (A - 1 + (A + 1) * c)
    a2 = A + 1 + (A - 1) * c - 2 * sA * alpha
    return b0 / a0, b1 / a0, b2 / a0, a1 / a0, a2 / a0


def _coef_highshelf(fc, gain_db, Q):
    A = 10.0 ** (gain_db / 40.0)
    w0 = 2.0 * math.pi * fc / FS
    alpha = np.sin(w0) / (2.0 * Q)
    c = np.cos(w0)
    sA = np.sqrt(A)
    b0 = A * (A + 1 + (A - 1) * c + 2 * sA * alpha)
    b1 = -2 * A * (A - 1 + (A + 1) * c)
    b2 = A * (A + 1 + (A - 1) * c - 2 * sA * alpha)
    a0 = A + 1 - (A - 1) * c + 2 * sA * alpha
    a1 = 2 * (A - 1 - (A + 1) * c)
    a2 = A + 1 - (A - 1) * c - 2 * sA * alpha
    return b0 / a0, b1 / a0, b2 / a0, a1 / a0, a2 / a0


def _coef_peak(fc, gain_db, Q):
    A = 10.0 ** (gain_db / 40.0)
    w0 = 2.0 * math.pi * fc / FS
    alpha = np.sin(w0) / (2.0 * Q)
    c = np.cos(w0)
    b0 = 1 + alpha * A; b1 = -2 * c; b2 = 1 - alpha * A
    a0 = 1 + alpha / A; a1 = -2 * c; a2 = 1 - alpha / A
    return b0 / a0, b1 / a0, b2 / a0, a1 / a0, a2 / a0


def _row_coeffs(p_row):
    bp = p_row[: N_BIQUADS * 3].reshape(N_BIQUADS, 3)
    bb_lo, bb_hi = BROADBAND_RANGE
    in_gain = 10.0 ** ((bb_lo + p_row[-2] * (bb_hi - bb_lo)) / 20.0)
    out_gain = 10.0 ** ((bb_lo + p_row[-1] * (bb_hi - bb_lo)) / 20.0)
    coefs = []
    for i in range(N_BIQUADS):
        fn, gn, qn = bp[i, 0], bp[i, 1], bp[i, 2]
        Q = _denorm_log(qn, *Q_RANGE)
        gain = GAIN_RANGE[0] + gn * (GAIN_RANGE[1] - GAIN_RANGE[0])
        if i == 0:
            cf = _coef_highpass(_denorm_log(fn, *HPF_FREQ_RANGE), Q)
        elif i == 15:
            cf = _coef_lowpass(_denorm_log(fn, *LPF_FREQ_RANGE), Q)
        elif i == 1:
            cf = _coef_lowshelf(_denorm_log(fn, *SHELF_FREQ_RANGE), gain, Q)
        elif i == 14:
            cf = _coef_highshelf(_denorm_log(fn, *SHELF_FREQ_RANGE), gain, Q)
        else:
            cf = _coef_peak(_denorm_log(fn, *PEAK_FREQ_RANGE), gain, Q)
        coefs.append(tuple(float(v) for v in cf))
    return coefs, float(in_gain), float(out_gain)


def _cascade_statespace(coefs, in_gain, out_gain):
    """Full-cascade state space (A, B, C, Dff), DF2-transposed per biquad."""
    n = NST
    A = np.zeros((n, n))
    B = np.zeros(n)
    d_u = in_gain
    c_u = np.zeros(n)
    for k, (b0, b1, b2, a1, a2) in enumerate(coefs):
        e1 = np.zeros(n); e1[2 * k] = 1.0
        e2 = np.zeros(n); e2[2 * k + 1] = 1.0
        d_y = b0 * d_u
        c_y = b0 * c_u + e1
        A[2 * k] = b1 * c_u - a1 * c_y + e2
        B[2 * k] = b1 * d_u - a1 * d_y
        A[2 * k + 1] = b2 * c_u - a2 * c_y
        B[2 * k + 1] = b2 * d_u - a2 * d_y
        d_u, c_u = d_y, c_y
    return A, B, out_gain * c_u, out_gain * d_u


def _row_device_matrices(p_row):
    """Float32 matrices for one batch row, laid out exactly as the device
    matmuls consume them."""
    coefs, ig, og = _row_coeffs(np.asarray(p_row, np.float64))
    A, B, Cv, Dff = _cascade_statespace(coefs, ig, og)
    n = NST
    # impulse response g[0..L-1]
    g = np.zeros(L)
    g[0] = Dff
    v = B.copy()
    for l in range(1, L):
        g[l] = Cv @ v
        v = A @ v
    # gt[j', q, l] = g[l - 128q - j']  (rhs of the y_zs matmul, K-quarter q)
    gt = np.zeros((128, NQ, L))
    for q in range(NQ):
        for jp in range(128):
            j = 128 * q + jp
            gt[jp, q, j:] = g[: L - j]
    # ut[q, j', i] = U[i, 128q + j'] = (A^{L-1-j} B)[i]  (lhsT of the D matmul)
    ut_full = np.zeros((L, n))
    w = B.copy()
    for j in range(L - 1, -1, -1):
        ut_full[j] = w
        w = A @ w
    ut = ut_full.reshape(NQ, 128, n).transpose(1, 0, 2)  # [j', q, i]
    # kt[i, l] = Kmat[l, i] = (C A^l)[i]  (rhs of the correction matmul)
    kt = np.zeros((n, L))
    kv = Cv.copy()
    for l in range(L):
        kt[:, l] = kv
        kv = kv @ A
    # scan matrices
    Abar = np.linalg.matrix_power(A, L)
    ups, downs = [], []
    M = Abar
    for _ in range(N_LEV):
        P4 = [np.linalg.matrix_power(M, p) for p in range(4)]
        # out rows 0:n = E_out; rows b*n:(b+1)*n = Sloc[b] for b=1..3
        up = np.zeros((4 * n, 4 * n))
        for k in range(4):
            up[0:n, k * n:(k + 1) * n] = P4[3 - k]
        for b in range(1, 4):
            for k in range(b):
                up[b * n:(b + 1) * n, k * n:(k + 1) * n] = P4[b - 1 - k]
        down = np.zeros((4 * n, n))
        for b in range(4):
            down[b * n:(b + 1) * n] = P4[b]
        ups.append(up.T)      # lhsT layout [K=in, M=out]
        downs.append(down.T)  # lhsT layout [K=j(32), M=(b,i)(128)]
        M = np.linalg.matrix_power(M, 4)
    f32 = np.float32
    return (gt.astype(f32), ut.astype(f32), kt.astype(f32),
            np.stack(ups).astype(f32), np.stack(downs).astype(f32))


def host_matrices(params):
    """params (16, 50) -> dict of stacked per-row device matrices."""
    gts, uts, kts, upss, dnss = [], [], [], [], []
    for b in range(params.shape[0]):
        gt, ut, kt, ups, dns = _row_device_matrices(params[b])
        gts.append(gt); uts.append(ut); kts.append(kt)
        upss.append(ups); dnss.append(dns)
    return dict(gt=np.stack(gts), ut=np.stack(uts), kt=np.stack(kts),
                scan_up=np.stack(upss), scan_down=np.stack(dnss))


# ---------------------------------------------------------------------------
# device program
# ---------------------------------------------------------------------------

_PROGRAM = None


def _emit(ctx, tc, nc, aps):
    import concourse.mybir as mybir
    from concourse.masks import make_identity

    f32 = mybir.dt.float32
    f32r = mybir.dt.float32r
    audio, gt, ut, kt, sup, sdn, yout = (
        aps["audio"], aps["gt"], aps["ut"], aps["kt"],
        aps["scan_up"], aps["scan_down"], aps["y"])

    const = ctx.enter_context(tc.tile_pool(name="const", bufs=1))
    data = ctx.enter_context(tc.tile_pool(name="data", bufs=1))
    pst = ctx.enter_context(tc.tile_pool(name="pst", bufs=3, space="PSUM"))
    psy = ctx.enter_context(tc.tile_pool(name="psy", bufs=5, space="PSUM"))
    stage = ctx.enter_context(tc.tile_pool(name="stage", bufs=4))

    ident = const.tile([128, 128], f32, name="ident", tag="ident")
    make_identity(nc, ident[:])

    # ---- input DMAs first (audio on the sync queue, block-granular) ----
    xin = [data.tile([128, NBLK, L], f32, name=f"xin{r}", tag=f"xin{r}")
           for r in range(2)]
    for r in range(2):
        asrc = audio[r].rearrange("(b p j) -> b p j", p=128, j=L)
        for blk in range(NBLK):
            nc.sync.dma_start(xin[r][:, blk, :], asrc[blk])

    # ---- constants on the scalar HWDGE queue ----
    gt_sb = [const.tile([128, NQ, L], f32r, name=f"gt{r}", tag=f"gt{r}")
             for r in range(2)]
    ut_sb = [const.tile([128, NQ, NST], f32, name=f"ut{r}", tag=f"ut{r}")
             for r in range(2)]
    kt_sb = [const.tile([NST, L], f32, name=f"kt{r}", tag=f"kt{r}")
             for r in range(2)]
    up_sb = [[const.tile([128, 128], f32, name=f"up{r}_{v}", tag=f"up{r}_{v}")
              for v in range(N_LEV)] for r in range(2)]
    dn_sb = [[const.tile([NST, 128], f32, name=f"dn{r}_{v}", tag=f"dn{r}_{v}")
              for v in range(N_LEV)] for r in range(2)]
    for r in range(2):
        nc.scalar.dma_start(ut_sb[r][:], ut[r])
        nc.scalar.dma_start(kt_sb[r][:], kt[r])
        for v in range(N_LEV):
            nc.scalar.dma_start(up_sb[r][v][:], sup[r, v])
            nc.scalar.dma_start(dn_sb[r][v][:], sdn[r, v])
    for r in range(2):
        nc.scalar.dma_start(gt_sb[r][:], gt[r].bitcast(f32r))

    # ---- transpose x into [j, c] layout (f32r rounding at the copy) ----
    xT = [data.tile([128, NQ, C], f32r, name=f"xT{r}", tag=f"xT{r}")
          for r in range(2)]
    for blk in range(NBLK):
        for r in range(2):
            for q in range(NQ):
                tp = pst.tile([128, 128], f32, name="tp", tag="pst")
                nc.tensor.transpose(tp[:], xin[r][:, blk, q * 128:(q + 1) * 128],
                                    ident[:])
                dst = xT[r][:, q, blk * 128:(blk + 1) * 128]
                if q % 2 == 0:
                    nc.vector.tensor_copy(dst, tp[:])
                else:
                    nc.scalar.copy(dst, tp[:])

    # ---- phase 1a: D = U @ x ----
    D_sb = [data.tile([NST, C], f32, name=f"D{r}", tag=f"D{r}") for r in range(2)]
    for r in range(2):
        dp = pst.tile([NST, C], f32, name="dp", tag="pst")
        for q in range(NQ):
            nc.tensor.matmul(dp[:], ut_sb[r][:, q, :],
                             xT[r][:, q, :].bitcast(f32),
                             start=(q == 0), stop=(q == NQ - 1))
        nc.vector.tensor_copy(D_sb[r][:], dp[:])

    # ---- phase 2: hierarchical scan (rows interleaved) ----
    E = [D_sb[r][:] for r in range(2)]
    upalls = [[], []]
    for lev, n_in in enumerate(SCAN_SIZES):
        n_g = n_in // 4
        for r in range(2):
            rhs4 = data.tile([128, n_g], f32, name=f"rhs4_{r}_{lev}", tag=f"rhs4_{r}_{lev}")
            Eg = E[r].rearrange("p (g b) -> p g b", b=4)
            for b in range(4):
                eng = nc.vector if b % 2 == 0 else nc.gpsimd
                eng.tensor_copy(rhs4[b * NST:(b + 1) * NST, :], Eg[:, :, b])
            upo = pst.tile([128, n_g], f32, name="upo", tag="pst")
            nc.tensor.matmul(upo[:], up_sb[r][lev][:], rhs4[:], start=True, stop=True)
            # rows 0:32 = E_out, rows 32:128 = Sloc[b=1..3]
            upall = data.tile([128, n_g], f32, name=f"upall_{r}_{lev}", tag=f"upall_{r}_{lev}")
            if r == 0:
                nc.vector.tensor_copy(upall[:], upo[:])
            else:
                nc.scalar.copy(upall[:], upo[:])
            upalls[r].append(upall)
            E[r] = upall[:NST, :]
    Sin = [None, None]
    for r in range(2):
        Sin_t = data.tile([NST, 2], f32, name=f"sintop_{r}", tag=f"sintop_{r}")
        nc.any.memset(Sin_t[:, 0:1], 0.0)
        nc.vector.tensor_copy(Sin_t[:, 1:2], E[r][:, 0:1])
        Sin[r] = Sin_t[:]
    for lev in range(N_LEV - 1, -1, -1):
        n_g = SCAN_SIZES[lev] // 4
        for r in range(2):
            prop = pst.tile([128, n_g], f32, name="prop", tag="pst")
            nc.tensor.matmul(prop[:], dn_sb[r][lev][:], Sin[r], start=True, stop=True)
            full = data.tile([128, n_g], f32, name=f"full_{r}_{lev}", tag=f"full_{r}_{lev}")
            nc.scalar.copy(full[:NST, :], prop[:NST, :])
            nc.vector.tensor_tensor(full[NST:2 * NST, :], prop[NST:2 * NST, :],
                                    upalls[r][lev][NST:2 * NST, :],
                                    op=mybir.AluOpType.add)
            nc.vector.tensor_tensor(full[2 * NST:, :], prop[2 * NST:, :],
                                    upalls[r][lev][2 * NST:, :],
                                    op=mybir.AluOpType.add)
            Snx = data.tile([NST, SCAN_SIZES[lev]], f32, name=f"snx_{r}_{lev}", tag=f"snx_{r}_{lev}")
            Sg = Snx[:].rearrange("p (g b) -> p g b", b=4)
            for b in range(4):
                eng = nc.vector if b % 2 == 0 else nc.gpsimd
                eng.tensor_copy(Sg[:, :, b], full[b * NST:(b + 1) * NST, :])
            Sin[r] = Snx[:]
    S_sb = Sin

    # ---- phase 1b: y_zs matmuls (emitted after the scan so the scheduler
    #      gives the scan chain priority and uses these to fill PE gaps) ----
    yps = {}
    for r in range(2):
        for blk in range(NBLK):
            cs = slice(blk * 128, (blk + 1) * 128)
            yp = psy.tile([128, L], f32, name="yp", tag="psy")
            for q in range(NQ):
                nc.tensor.matmul(yp[:], xT[r][:, q, cs], gt_sb[r][:, q, :],
                                 start=(q == 0), stop=False)
            yps[(r, blk)] = yp

    # ---- phase 3: corrections accumulate into the y PSUM, stage, DMA out ----
    for r in range(2):
        ydst = yout[r].rearrange("(b p l) -> b p l", p=128, l=L)
        for blk in range(NBLK):
            cs = slice(blk * 128, (blk + 1) * 128)
            yp = yps[(r, blk)]
            nc.tensor.matmul(yp[:], S_sb[r][:, cs], kt_sb[r][:],
                             start=False, stop=True)
            ystage = stage.tile([128, L], f32, name="ystage", tag="ystage")
            if blk % 2 == 0:
                nc.vector.tensor_copy(ystage[:], yp[:])
            else:
                nc.scalar.copy(ystage[:], yp[:])
            dq = nc.sync if blk % 2 == 0 else nc.scalar
            dq.dma_start(ydst[blk], ystage[:])


def build_program():
    from contextlib import ExitStack

    import concourse.mybir as mybir
    import concourse.tile as tile
    from concourse import bacc

    nc = bacc.Bacc("TRN2", target_bir_lowering=False, debug=False,
                   num_devices=N_CORES)
    f32 = mybir.dt.float32
    aps = dict(
        audio=nc.dram_tensor("audio", [2, T], f32, kind="ExternalInput").ap(),
        gt=nc.dram_tensor("gt", [2, 128, NQ, L], f32, kind="ExternalInput").ap(),
        ut=nc.dram_tensor("ut", [2, 128, NQ, NST], f32, kind="ExternalInput").ap(),
        kt=nc.dram_tensor("kt", [2, NST, L], f32, kind="ExternalInput").ap(),
        scan_up=nc.dram_tensor("scan_up", [2, N_LEV, 128, 128], f32,
                               kind="ExternalInput").ap(),
        scan_down=nc.dram_tensor("scan_down", [2, N_LEV, NST, 128], f32,
                                 kind="ExternalInput").ap(),
        y=nc.dram_tensor("y", [2, T], f32, kind="ExternalOutput").ap(),
    )
    with tile.TileContext(nc) as tc:
        with ExitStack() as ctx:
            _emit(ctx, tc, nc, aps)
    nc.compile()
    return nc


def _get_program():
    global _PROGRAM
    if _PROGRAM is None:
        _PROGRAM = build_program()
    return _PROGRAM


def make_in_maps(audio, params):
    audio = np.ascontiguousarray(np.asarray(audio, np.float32))
    mats = host_matrices(np.asarray(params, np.float64))
    in_maps = []
    for i in range(N_CORES):
        s = slice(2 * i, 2 * i + 2)
        in_maps.append({
            "audio": audio[s],
            "gt": mats["gt"][s],
            "ut": mats["ut"][s],
            "kt": mats["kt"][s],
            "scan_up": mats["scan_up"][s],
            "scan_down": mats["scan_down"][s],
        })
    return in_maps


def kernel(audio, params):
    from concourse.bass_utils import run_bass_kernel_spmd

    nc = _get_program()
    in_maps = make_in_maps(audio, params)
    res = run_bass_kernel_spmd(nc, in_maps, list(range(N_CORES)))
    return np.concatenate([res.results[i]["y"] for i in range(N_CORES)],
                          axis=0).astype(np.float32)


# revision 21
# speedup vs baseline: 1.0015x; 1.0015x over previous
"""Trainium2 Bass kernel for nn_DifferentiableBiquadChain.

16 cascaded biquads over (16, 262144) audio. The whole cascade is one LTI
system with a 32-dim state; we decompose each batch row's sequence into 2048
chunks of 128 samples and compute:

  phase 1a: per-chunk end-state contribution  D_c = U @ x_c        (TensorE)
  phase 2:  boundary-state scan S_{c+1} = A^128 S_c + D_c, done as a
            radix-4 hierarchical matmul scan over precomputed A-powers
  phase 1b: per-chunk zero-state response  y_zs = Toeplitz(g) @ x_c (TensorE)
  phase 3:  correction y_c = y_zs + Kmat @ S_c, PSUM-accumulated

All matrices (impulse response g, U, Kmat, scan-level A-power blocks) are
precomputed on host in float64 from `params`; the device does only fp32
matmuls + copies. Data-parallel: 2 batch rows per core on 8 cores.
"""
import math

import numpy as np

FS = 96000.0
N_BIQUADS = 16
HPF_FREQ_RANGE = (20.0, 500.0)
LPF_FREQ_RANGE = (5000.0, 20000.0)
SHELF_FREQ_RANGE = (50.0, 16000.0)
PEAK_FREQ_RANGE = (100.0, 15000.0)
Q_RANGE = (0.5, 16.0)
GAIN_RANGE = (-24.0, 24.0)
BROADBAND_RANGE = (-60.0, 0.0)

T = 262144
L = 512          # chunk length
NQ = L // 128    # K-quarters per chunk
C = T // L       # 512 chunks per row
NBLK = C // 128  # 4 chunk-blocks per row
NST = 2 * N_BIQUADS   # 32 state dims
ROWS_PER_CORE = 2
N_CORES = 8
SCAN_SIZES = [512, 128, 32, 8]  # radix-4 upsweep input sizes
N_LEV = len(SCAN_SIZES)


# ---------------------------------------------------------------------------
# host-side math (float64)
# ---------------------------------------------------------------------------

def _denorm_log(norm, lo, hi):
    return np.exp(math.log(lo) + norm * (math.log(hi) - math.log(lo)))


def _coef_highpass(fc, Q):
    w0 = 2.0 * math.pi * fc / FS
    alpha = np.sin(w0) / (2.0 * Q)
    c = np.cos(w0)
    b0 = (1 + c) / 2; b1 = -(1 + c); b2 = (1 + c) / 2
    a0 = 1 + alpha; a1 = -2 * c; a2 = 1 - alpha
    return b0 / a0, b1 / a0, b2 / a0, a1 / a0, a2 / a0


def _coef_lowpass(fc, Q):
    w0 = 2.0 * math.pi * fc / FS
    alpha = np.sin(w0) / (2.0 * Q)
    c = np.cos(w0)
    b0 = (1 - c) / 2; b1 = 1 - c; b2 = (1 - c) / 2
    a0 = 1 + alpha; a1 = -2 * c; a2 = 1 - alpha
    return b0 / a0, b1 / a0, b2 / a0, a1 / a0, a2 / a0


def _coef_lowshelf(fc, gain_db, Q):
    A = 10.0 ** (gain_db / 40.0)
    w0 = 2.0 * math.pi * fc / FS
    alpha = np.sin(w0) / (2.0 * Q)
    c = np.cos(w0)
    sA = np.sqrt(A)
    b0 = A * (A + 1 - (A - 1) * c + 2 * sA * alpha)
    b1 = 2 * A * (A - 1 - (A + 1) * c)
    b2 = A * (A + 1 - (A - 1) * c - 2 * sA * alpha)
    a0 = A + 1 + (A - 1) * c + 2 * sA * alpha
    a1 = -2 * (A - 1 + (A + 1) * c)
    a2 = A + 1 + (A - 1) * c - 2 * sA * alpha
    return b0 / a0, b1 / a0, b2 / a0, a1 / a0, a2 / a0


def _coef_highshelf(fc, gain_db, Q):
    A = 10.0 ** (gain_db / 40.0)
    w0 = 2.0 * math.pi * fc / FS
    alpha = np.sin(w0) / (2.0 * Q)
    c = np.cos(w0)
    sA = np.sqrt(A)
    b0 = A * (A + 1 + (A - 1) * c + 2 * sA * alpha)
    b1 = -2 * A * (A - 1 + (A + 1) * c)
    b2 = A * (A + 1 + (A - 1) * c - 2 * sA * alpha)
    a0 = A + 1 - (A - 1) * c + 2 * sA * alpha
    a1 = 2 * (A - 1 - (A + 1) * c)
    a2 = A + 1 - (A - 1) * c - 2 * sA * alpha
    return b0 / a0, b1 / a0, b2 / a0, a1 / a0, a2 / a0


def _coef_peak(fc, gain_db, Q):
    A = 10.0 ** (gain_db / 40.0)
    w0 = 2.0 * math.pi * fc / FS
    alpha = np.sin(w0) / (2.0 * Q)
    c = np.cos(w0)
    b0 = 1 + alpha * A; b1 = -2 * c; b2 = 1 - alpha * A
    a0 = 1 + alpha / A; a1 = -2 * c; a2 = 1 - alpha / A
    return b0 / a0, b1 / a0, b2 / a0, a1 / a0, a2 / a0


def _row_coeffs(p_row):
    bp = p_row[: N_BIQUADS * 3].reshape(N_BIQUADS, 3)
    bb_lo, bb_hi = BROADBAND_RANGE
    in_gain = 10.0 ** ((bb_lo + p_row[-2] * (bb_hi - bb_lo)) / 20.0)
    out_gain = 10.0 ** ((bb_lo + p_row[-1] * (bb_hi - bb_lo)) / 20.0)
    coefs = []
    for i in range(N_BIQUADS):
        fn, gn, qn = bp[i, 0], bp[i, 1], bp[i, 2]
        Q = _denorm_log(qn, *Q_RANGE)
        gain = GAIN_RANGE[0] + gn * (GAIN_RANGE[1] - GAIN_RANGE[0])
        if i == 0:
            cf = _coef_highpass(_denorm_log(fn, *HPF_FREQ_RANGE), Q)
        elif i == 15:
            cf = _coef_lowpass(_denorm_log(fn, *LPF_FREQ_RANGE), Q)
        elif i == 1:
            cf = _coef_lowshelf(_denorm_log(fn, *SHELF_FREQ_RANGE), gain, Q)
        elif i == 14:
            cf = _coef_highshelf(_denorm_log(fn, *SHELF_FREQ_RANGE), gain, Q)
        else:
            cf = _coef_peak(_denorm_log(fn, *PEAK_FREQ_RANGE), gain, Q)
        coefs.append(tuple(float(v) for v in cf))
    return coefs, float(in_gain), float(out_gain)


def _cascade_statespace(coefs, in_gain, out_gain):
    """Full-cascade state space (A, B, C, Dff), DF2-transposed per biquad."""
    n = NST
    A = np.zeros((n, n))
    B = np.zeros(n)
    d_u = in_gain
    c_u = np.zeros(n)
    for k, (b0, b1, b2, a1, a2) in enumerate(coefs):
        e1 = np.zeros(n); e1[2 * k] = 1.0
        e2 = np.zeros(n); e2[2 * k + 1] = 1.0
        d_y = b0 * d_u
        c_y = b0 * c_u + e1
        A[2 * k] = b1 * c_u - a1 * c_y + e2
        B[2 * k] = b1 * d_u - a1 * d_y
        A[2 * k + 1] = b2 * c_u - a2 * c_y
        B[2 * k + 1] = b2 * d_u - a2 * d_y
        d_u, c_u = d_y, c_y
    return A, B, out_gain * c_u, out_gain * d_u


def _row_device_matrices(p_row):
    """Float32 matrices for one batch row, laid out exactly as the device
    matmuls consume them."""
    coefs, ig, og = _row_coeffs(np.asarray(p_row, np.float64))
    A, B, Cv, Dff = _cascade_statespace(coefs, ig, og)
    n = NST
    # impulse response g[0..L-1]
    g = np.zeros(L)
    g[0] = Dff
    v = B.copy()
    for l in range(1, L):
        g[l] = Cv @ v
        v = A @ v
    # gt[j', q, l] = g[l - 128q - j']  (rhs of the y_zs matmul, K-quarter q)
    gt = np.zeros((128, NQ, L))
    for q in range(NQ):
        for jp in range(128):
            j = 128 * q + jp
            gt[jp, q, j:] = g[: L - j]
    # ut[q, j', i] = U[i, 128q + j'] = (A^{L-1-j} B)[i]  (lhsT of the D matmul)
    ut_full = np.zeros((L, n))
    w = B.copy()
    for j in range(L - 1, -1, -1):
        ut_full[j] = w
        w = A @ w
    ut = ut_full.reshape(NQ, 128, n).transpose(1, 0, 2)  # [j', q, i]
    # kt[i, l] = Kmat[l, i] = (C A^l)[i]  (rhs of the correction matmul)
    kt = np.zeros((n, L))
    kv = Cv.copy()
    for l in range(L):
        kt[:, l] = kv
        kv = kv @ A
    # scan matrices
    Abar = np.linalg.matrix_power(A, L)
    ups, downs = [], []
    M = Abar
    for _ in range(N_LEV):
        P4 = [np.linalg.matrix_power(M, p) for p in range(4)]
        # out rows 0:n = E_out; rows b*n:(b+1)*n = Sloc[b] for b=1..3
        up = np.zeros((4 * n, 4 * n))
        for k in range(4):
            up[0:n, k * n:(k + 1) * n] = P4[3 - k]
        for b in range(1, 4):
            for k in range(b):
                up[b * n:(b + 1) * n, k * n:(k + 1) * n] = P4[b - 1 - k]
        down = np.zeros((4 * n, n))
        for b in range(4):
            down[b * n:(b + 1) * n] = P4[b]
        ups.append(up.T)      # lhsT layout [K=in, M=out]
        downs.append(down.T)  # lhsT layout [K=j(32), M=(b,i)(128)]
        M = np.linalg.matrix_power(M, 4)
    f32 = np.float32
    return (gt.astype(f32), ut.astype(f32), kt.astype(f32),
            np.stack(ups).astype(f32), np.stack(downs).astype(f32))


def host_matrices(params):
    """params (16, 50) -> dict of stacked per-row device matrices."""
    gts, uts, kts, upss, dnss = [], [], [], [], []
    for b in range(params.shape[0]):
        gt, ut, kt, ups, dns = _row_device_matrices(params[b])
        gts.append(gt); uts.append(ut); kts.append(kt)
        upss.append(ups); dnss.append(dns)
    return dict(gt=np.stack(gts), ut=np.stack(uts), kt=np.stack(kts),
                scan_up=np.stack(upss), scan_down=np.stack(dnss))


# ---------------------------------------------------------------------------
# device program
# ---------------------------------------------------------------------------

_PROGRAM = None


def _emit(ctx, tc, nc, aps):
    import concourse.mybir as mybir
    from concourse.masks import make_identity

    f32 = mybir.dt.float32
    f32r = mybir.dt.float32r
    audio, gt, ut, kt, sup, sdn, yout = (
        aps["audio"], aps["gt"], aps["ut"], aps["kt"],
        aps["scan_up"], aps["scan_down"], aps["y"])

    const = ctx.enter_context(tc.tile_pool(name="const", bufs=1))
    data = ctx.enter_context(tc.tile_pool(name="data", bufs=1))
    pst = ctx.enter_context(tc.tile_pool(name="pst", bufs=3, space="PSUM"))
    psy = ctx.enter_context(tc.tile_pool(name="psy", bufs=3, space="PSUM"))
    psc = ctx.enter_context(tc.tile_pool(name="psc", bufs=2, space="PSUM"))
    stage = ctx.enter_context(tc.tile_pool(name="stage", bufs=16))

    ident = const.tile([128, 128], f32, name="ident", tag="ident")
    make_identity(nc, ident[:])

    # ---- input DMAs first (audio on the sync queue, block-granular) ----
    xin = [data.tile([128, NBLK, L], f32, name=f"xin{r}", tag=f"xin{r}")
           for r in range(2)]
    for r in range(2):
        asrc = audio[r].rearrange("(b p j) -> b p j", p=128, j=L)
        for blk in range(NBLK):
            nc.sync.dma_start(xin[r][:, blk, :], asrc[blk])

    # ---- constants on the scalar HWDGE queue ----
    gt_sb = [const.tile([128, NQ, L], f32r, name=f"gt{r}", tag=f"gt{r}")
             for r in range(2)]
    ut_sb = [const.tile([128, NQ, NST], f32, name=f"ut{r}", tag=f"ut{r}")
             for r in range(2)]
    kt_sb = [const.tile([NST, L], f32, name=f"kt{r}", tag=f"kt{r}")
             for r in range(2)]
    up_sb = [[const.tile([128, 128], f32, name=f"up{r}_{v}", tag=f"up{r}_{v}")
              for v in range(N_LEV)] for r in range(2)]
    dn_sb = [[const.tile([NST, 128], f32, name=f"dn{r}_{v}", tag=f"dn{r}_{v}")
              for v in range(N_LEV)] for r in range(2)]
    for r in range(2):
        nc.scalar.dma_start(ut_sb[r][:], ut[r])
        nc.scalar.dma_start(kt_sb[r][:], kt[r])
        for v in range(N_LEV):
            nc.scalar.dma_start(up_sb[r][v][:], sup[r, v])
            nc.scalar.dma_start(dn_sb[r][v][:], sdn[r, v])
    for r in range(2):
        nc.scalar.dma_start(gt_sb[r][:], gt[r].bitcast(f32r))

    # ---- transpose x into [j, c] layout (f32r rounding at the copy) ----
    xT = [data.tile([128, NQ, C], f32r, name=f"xT{r}", tag=f"xT{r}")
          for r in range(2)]
    for blk in range(NBLK):
        for r in range(2):
            for q in range(NQ):
                tp = pst.tile([128, 128], f32, name="tp", tag="pst")
                nc.tensor.transpose(tp[:], xin[r][:, blk, q * 128:(q + 1) * 128],
                                    ident[:])
                dst = xT[r][:, q, blk * 128:(blk + 1) * 128]
                if q % 2 == 0:
                    nc.vector.tensor_copy(dst, tp[:])
                else:
                    nc.scalar.copy(dst, tp[:])

    # ---- phase 1a: D = U @ x ----
    D_sb = [data.tile([NST, C], f32, name=f"D{r}", tag=f"D{r}") for r in range(2)]
    for r in range(2):
        dp = pst.tile([NST, C], f32, name="dp", tag="pst")
        for q in range(NQ):
            nc.tensor.matmul(dp[:], ut_sb[r][:, q, :],
                             xT[r][:, q, :].bitcast(f32),
                             start=(q == 0), stop=(q == NQ - 1))
        nc.vector.tensor_copy(D_sb[r][:], dp[:])

    # ---- phase 2 (scan) and phase 1b (y_zs) interleaved: one scan step,
    #      then one y-block, so the in-order PE stream never stalls long ----
    E = [D_sb[r][:] for r in range(2)]
    upalls = [[], []]
    Sin = [None, None]
    S_sb = [None, None]

    def up_step(r, lev):
        n_in = SCAN_SIZES[lev]
        n_g = n_in // 4
        rhs4 = data.tile([128, n_g], f32, name=f"rhs4_{r}_{lev}", tag=f"rhs4_{r}_{lev}")
        Eg = E[r].rearrange("p (g b) -> p g b", b=4)
        for b in range(4):
            eng = nc.vector if b % 2 == 0 else nc.gpsimd
            eng.tensor_copy(rhs4[b * NST:(b + 1) * NST, :], Eg[:, :, b])
        upo = pst.tile([128, n_g], f32, name="upo", tag="pst")
        nc.tensor.matmul(upo[:], up_sb[r][lev][:], rhs4[:], start=True, stop=True)
        # rows 0:32 = E_out, rows 32:128 = Sloc[b=1..3]
        upall = data.tile([128, n_g], f32, name=f"upall_{r}_{lev}", tag=f"upall_{r}_{lev}")
        if r == 0:
            nc.vector.tensor_copy(upall[:], upo[:])
        else:
            nc.scalar.copy(upall[:], upo[:])
        upalls[r].append(upall)
        E[r] = upall[:NST, :]

    def top_step(r):
        Sin_t = data.tile([NST, 2], f32, name=f"sintop_{r}", tag=f"sintop_{r}")
        nc.any.memset(Sin_t[:, 0:1], 0.0)
        nc.vector.tensor_copy(Sin_t[:, 1:2], E[r][:, 0:1])
        Sin[r] = Sin_t[:]

    def down_step(r, lev):
        n_g = SCAN_SIZES[lev] // 4
        prop = pst.tile([128, n_g], f32, name="prop", tag="pst")
        nc.tensor.matmul(prop[:], dn_sb[r][lev][:], Sin[r], start=True, stop=True)
        full = data.tile([128, n_g], f32, name=f"full_{r}_{lev}", tag=f"full_{r}_{lev}")
        nc.scalar.copy(full[:NST, :], prop[:NST, :])
        nc.vector.tensor_tensor(full[NST:2 * NST, :], prop[NST:2 * NST, :],
                                upalls[r][lev][NST:2 * NST, :],
                                op=mybir.AluOpType.add)
        nc.vector.tensor_tensor(full[2 * NST:, :], prop[2 * NST:, :],
                                upalls[r][lev][2 * NST:, :],
                                op=mybir.AluOpType.add)
        Snx = data.tile([NST, SCAN_SIZES[lev]], f32, name=f"snx_{r}_{lev}", tag=f"snx_{r}_{lev}")
        Sg = Snx[:].rearrange("p (g b) -> p g b", b=4)
        for b in range(4):
            eng = nc.vector if b % 2 == 0 else nc.gpsimd
            eng.tensor_copy(Sg[:, :, b], full[b * NST:(b + 1) * NST, :])
        Sin[r] = Snx[:]
        if lev == 0:
            S_sb[r] = Snx[:]

    ystages = {}

    def y_block(r, blk):
        cs = slice(blk * 128, (blk + 1) * 128)
        yp = psy.tile([128, L], f32, name="yp", tag="psy")
        for q in range(NQ):
            nc.tensor.matmul(yp[:], xT[r][:, q, cs], gt_sb[r][:, q, :],
                             start=(q == 0), stop=(q == NQ - 1))
        ystage = stage.tile([128, L], f32, name="ystage", tag="ystage")
        if blk % 2 == 0:
            nc.vector.tensor_copy(ystage[:], yp[:])
        else:
            nc.scalar.copy(ystage[:], yp[:])
        ystages[(r, blk)] = ystage

    scan_steps = []
    for lev in range(N_LEV):
        for r in range(2):
            scan_steps.append(lambda r=r, lev=lev: up_step(r, lev))
    for r in range(2):
        scan_steps.append(lambda r=r: top_step(r))
    for lev in range(N_LEV - 1, -1, -1):
        for r in range(2):
            scan_steps.append(lambda r=r, lev=lev: down_step(r, lev))

    y_list = [(r, blk) for blk in range(NBLK) for r in range(2)]
    yi = 0
    for si, step in enumerate(scan_steps):
        step()
        if yi < len(y_list):
            y_block(*y_list[yi]); yi += 1
    while yi < len(y_list):
        y_block(*y_list[yi]); yi += 1

    # ---- phase 3: corrections, add, DMA out ----
    for blk in range(NBLK):
        for r in range(2):
            cs = slice(blk * 128, (blk + 1) * 128)
            ydst = yout[r].rearrange("(b p l) -> b p l", p=128, l=L)
            cp = psc.tile([128, L], f32, name="cp", tag="psc")
            nc.tensor.matmul(cp[:], S_sb[r][:, cs], kt_sb[r][:],
                             start=True, stop=True)
            ystage = ystages[(r, blk)]
            nc.vector.tensor_tensor(ystage[:], ystage[:], cp[:],
                                    op=mybir.AluOpType.add)
            dq = nc.sync if blk % 2 == 0 else nc.scalar
            dq.dma_start(ydst[blk], ystage[:])


def build_program():
    from contextlib import ExitStack

    import concourse.mybir as mybir
    import concourse.tile as tile
    from concourse import bacc

    nc = bacc.Bacc("TRN2", target_bir_lowering=False, debug=False,
                   num_devices=N_CORES)
    f32 = mybir.dt.float32
    aps = dict(
        audio=nc.dram_tensor("audio", [2, T], f32, kind="ExternalInput").ap(),
        gt=nc.dram_tensor("gt", [2, 128, NQ, L], f32, kind="ExternalInput").ap(),
        ut=nc.dram_tensor("ut", [2, 128, NQ, NST], f32, kind="ExternalInput").ap(),
        kt=nc.dram_tensor("kt", [2, NST, L], f32, kind="ExternalInput").ap(),
        scan_up=nc.dram_tensor("scan_up", [2, N_LEV, 128, 128], f32,
                               kind="ExternalInput").ap(),
        scan_down=nc.dram_tensor("scan_down", [2, N_LEV, NST, 128], f32,
                                 kind="ExternalInput").ap(),
        y=nc.dram_tensor("y", [2, T], f32, kind="ExternalOutput").ap(),
    )
    with tile.TileContext(nc) as tc:
        with ExitStack() as ctx:
            _emit(ctx, tc, nc, aps)
    nc.compile()
    return nc


def _get_program():
    global _PROGRAM
    if _PROGRAM is None:
        _PROGRAM = build_program()
    return _PROGRAM


def make_in_maps(audio, params):
    audio = np.ascontiguousarray(np.asarray(audio, np.float32))
    mats = host_matrices(np.asarray(params, np.float64))
    in_maps = []
    for i in range(N_CORES):
        s = slice(2 * i, 2 * i + 2)
        in_maps.append({
            "audio": audio[s],
            "gt": mats["gt"][s],
            "ut": mats["ut"][s],
            "kt": mats["kt"][s],
            "scan_up": mats["scan_up"][s],
            "scan_down": mats["scan_down"][s],
        })
    return in_maps


def kernel(audio, params):
    from concourse.bass_utils import run_bass_kernel_spmd

    nc = _get_program()
    in_maps = make_in_maps(audio, params)
    res = run_bass_kernel_spmd(nc, in_maps, list(range(N_CORES)))
    return np.concatenate([res.results[i]["y"] for i in range(N_CORES)],
                          axis=0).astype(np.float32)


# revision 23
# speedup vs baseline: 1.0340x; 1.0325x over previous
"""Trainium2 Bass kernel for nn_DifferentiableBiquadChain.

16 cascaded biquads over (16, 262144) audio. The whole cascade is one LTI
system with a 32-dim state; we decompose each batch row's sequence into 2048
chunks of 128 samples and compute:

  phase 1a: per-chunk end-state contribution  D_c = U @ x_c        (TensorE)
  phase 2:  boundary-state scan S_{c+1} = A^128 S_c + D_c, done as a
            radix-4 hierarchical matmul scan over precomputed A-powers
  phase 1b: per-chunk zero-state response  y_zs = Toeplitz(g) @ x_c (TensorE)
  phase 3:  correction y_c = y_zs + Kmat @ S_c, PSUM-accumulated

All matrices (impulse response g, U, Kmat, scan-level A-power blocks) are
precomputed on host in float64 from `params`; the device does only fp32
matmuls + copies. Data-parallel: 2 batch rows per core on 8 cores.
"""
import math

import numpy as np

FS = 96000.0
N_BIQUADS = 16
HPF_FREQ_RANGE = (20.0, 500.0)
LPF_FREQ_RANGE = (5000.0, 20000.0)
SHELF_FREQ_RANGE = (50.0, 16000.0)
PEAK_FREQ_RANGE = (100.0, 15000.0)
Q_RANGE = (0.5, 16.0)
GAIN_RANGE = (-24.0, 24.0)
BROADBAND_RANGE = (-60.0, 0.0)

T = 262144
L = 512          # chunk length
NQ = L // 128    # K-quarters per chunk
C = T // L       # 512 chunks per row
NBLK = C // 128  # 4 chunk-blocks per row
NST = 2 * N_BIQUADS   # 32 state dims
ROWS_PER_CORE = 2
N_CORES = 8
SCAN_SIZES = [512, 128, 32, 8]  # radix-4 upsweep input sizes
N_LEV = len(SCAN_SIZES)


# ---------------------------------------------------------------------------
# host-side math (float64)
# ---------------------------------------------------------------------------

def _denorm_log(norm, lo, hi):
    return np.exp(math.log(lo) + norm * (math.log(hi) - math.log(lo)))


def _coef_highpass(fc, Q):
    w0 = 2.0 * math.pi * fc / FS
    alpha = np.sin(w0) / (2.0 * Q)
    c = np.cos(w0)
    b0 = (1 + c) / 2; b1 = -(1 + c); b2 = (1 + c) / 2
    a0 = 1 + alpha; a1 = -2 * c; a2 = 1 - alpha
    return b0 / a0, b1 / a0, b2 / a0, a1 / a0, a2 / a0


def _coef_lowpass(fc, Q):
    w0 = 2.0 * math.pi * fc / FS
    alpha = np.sin(w0) / (2.0 * Q)
    c = np.cos(w0)
    b0 = (1 - c) / 2; b1 = 1 - c; b2 = (1 - c) / 2
    a0 = 1 + alpha; a1 = -2 * c; a2 = 1 - alpha
    return b0 / a0, b1 / a0, b2 / a0, a1 / a0, a2 / a0


def _coef_lowshelf(fc, gain_db, Q):
    A = 10.0 ** (gain_db / 40.0)
    w0 = 2.0 * math.pi * fc / FS
    alpha = np.sin(w0) / (2.0 * Q)
    c = np.cos(w0)
    sA = np.sqrt(A)
    b0 = A * (A + 1 - (A - 1) * c + 2 * sA * alpha)
    b1 = 2 * A * (A - 1 - (A + 1) * c)
    b2 = A * (A + 1 - (A - 1) * c - 2 * sA * alpha)
    a0 = A + 1 + (A - 1) * c + 2 * sA * alpha
    a1 = -2 * (A - 1 + (A + 1) * c)
    a2 = A + 1 + (A - 1) * c - 2 * sA * alpha
    return b0 / a0, b1 / a0, b2 / a0, a1 / a0, a2 / a0


def _coef_highshelf(fc, gain_db, Q):
    A = 10.0 ** (gain_db / 40.0)
    w0 = 2.0 * math.pi * fc / FS
    alpha = np.sin(w0) / (2.0 * Q)
    c = np.cos(w0)
    sA = np.sqrt(A)
    b0 = A * (A + 1 + (A - 1) * c + 2 * sA * alpha)
    b1 = -2 * A * (A - 1 + (A + 1) * c)
    b2 = A * (A + 1 + (A - 1) * c - 2 * sA * alpha)
    a0 = A + 1 - (A - 1) * c + 2 * sA * alpha
    a1 = 2 * (A - 1 - (A + 1) * c)
    a2 = A + 1 - (A - 1) * c - 2 * sA * alpha
    return b0 / a0, b1 / a0, b2 / a0, a1 / a0, a2 / a0


def _coef_peak(fc, gain_db, Q):
    A = 10.0 ** (gain_db / 40.0)
    w0 = 2.0 * math.pi * fc / FS
    alpha = np.sin(w0) / (2.0 * Q)
    c = np.cos(w0)
    b0 = 1 + alpha * A; b1 = -2 * c; b2 = 1 - alpha * A
    a0 = 1 + alpha / A; a1 = -2 * c; a2 = 1 - alpha / A
    return b0 / a0, b1 / a0, b2 / a0, a1 / a0, a2 / a0


def _row_coeffs(p_row):
    bp = p_row[: N_BIQUADS * 3].reshape(N_BIQUADS, 3)
    bb_lo, bb_hi = BROADBAND_RANGE
    in_gain = 10.0 ** ((bb_lo + p_row[-2] * (bb_hi - bb_lo)) / 20.0)
    out_gain = 10.0 ** ((bb_lo + p_row[-1] * (bb_hi - bb_lo)) / 20.0)
    coefs = []
    for i in range(N_BIQUADS):
        fn, gn, qn = bp[i, 0], bp[i, 1], bp[i, 2]
        Q = _denorm_log(qn, *Q_RANGE)
        gain = GAIN_RANGE[0] + gn * (GAIN_RANGE[1] - GAIN_RANGE[0])
        if i == 0:
            cf = _coef_highpass(_denorm_log(fn, *HPF_FREQ_RANGE), Q)
        elif i == 15:
            cf = _coef_lowpass(_denorm_log(fn, *LPF_FREQ_RANGE), Q)
        elif i == 1:
            cf = _coef_lowshelf(_denorm_log(fn, *SHELF_FREQ_RANGE), gain, Q)
        elif i == 14:
            cf = _coef_highshelf(_denorm_log(fn, *SHELF_FREQ_RANGE), gain, Q)
        else:
            cf = _coef_peak(_denorm_log(fn, *PEAK_FREQ_RANGE), gain, Q)
        coefs.append(tuple(float(v) for v in cf))
    return coefs, float(in_gain), float(out_gain)


def _cascade_statespace(coefs, in_gain, out_gain):
    """Full-cascade state space (A, B, C, Dff), DF2-transposed per biquad."""
    n = NST
    A = np.zeros((n, n))
    B = np.zeros(n)
    d_u = in_gain
    c_u = np.zeros(n)
    for k, (b0, b1, b2, a1, a2) in enumerate(coefs):
        e1 = np.zeros(n); e1[2 * k] = 1.0
        e2 = np.zeros(n); e2[2 * k + 1] = 1.0
        d_y = b0 * d_u
        c_y = b0 * c_u + e1
        A[2 * k] = b1 * c_u - a1 * c_y + e2
        B[2 * k] = b1 * d_u - a1 * d_y
        A[2 * k + 1] = b2 * c_u - a2 * c_y
        B[2 * k + 1] = b2 * d_u - a2 * d_y
        d_u, c_u = d_y, c_y
    return A, B, out_gain * c_u, out_gain * d_u


def _modal_realization(A, B, Cv):
    """Similarity-transform to a real modal (2x2 block-diagonal) basis with
    input/output-balanced mode scaling. Same transfer function; states become
    well-scaled so f32r rounding in D = U@x and y_corr = K@S stays benign."""
    n = A.shape[0]
    w, V = np.linalg.eig(A)
    T = np.zeros((n, n))
    used = np.zeros(len(w), bool)
    col = 0
    for i in range(len(w)):
        if used[i]:
            continue
        li, vi = w[i], V[:, i]
        if abs(li.imag) < 1e-12:
            T[:, col] = vi.real / np.linalg.norm(vi.real)
            used[i] = True
            col += 1
            continue
        # find conjugate partner
        j = min((jj for jj in range(len(w)) if not used[jj] and jj != i),
                key=lambda jj: abs(w[jj] - np.conj(li)))
        re, im = vi.real, vi.imag
        nrm = max(np.linalg.norm(re), np.linalg.norm(im))
        T[:, col] = re / nrm
        T[:, col + 1] = im / nrm
        used[i] = used[j] = True
        col += 2
    Ti = np.linalg.inv(T)
    A2, B2, C2 = Ti @ A @ T, Ti @ B, Cv @ T
    # balance per-state input vs output weighting
    s = np.sqrt((np.abs(B2) + 1e-30) / (np.abs(C2) + 1e-30))
    s = np.clip(s, 1e-6, 1e6)
    A2 = A2 * (s[None, :] / s[:, None])
    B2 = B2 / s
    C2 = C2 * s
    return A2, B2, C2


def _row_device_matrices(p_row):
    """Float32 matrices for one batch row, laid out exactly as the device
    matmuls consume them."""
    coefs, ig, og = _row_coeffs(np.asarray(p_row, np.float64))
    A, B, Cv, Dff = _cascade_statespace(coefs, ig, og)
    A, B, Cv = _modal_realization(A, B, Cv)
    n = NST
    # impulse response g[0..L-1]
    g = np.zeros(L)
    g[0] = Dff
    v = B.copy()
    for l in range(1, L):
        g[l] = Cv @ v
        v = A @ v
    # gt[j', q, l] = g[l - 128q - j']  (rhs of the y_zs matmul, K-quarter q)
    gt = np.zeros((128, NQ, L))
    for q in range(NQ):
        for jp in range(128):
            j = 128 * q + jp
            gt[jp, q, j:] = g[: L - j]
    # ut[q, j', i] = U[i, 128q + j'] = (A^{L-1-j} B)[i]  (lhsT of the D matmul)
    ut_full = np.zeros((L, n))
    w = B.copy()
    for j in range(L - 1, -1, -1):
        ut_full[j] = w
        w = A @ w
    ut = ut_full.reshape(NQ, 128, n).transpose(1, 0, 2)  # [j', q, i]
    # kt[i, l] = Kmat[l, i] = (C A^l)[i]  (rhs of the correction matmul)
    kt = np.zeros((n, L))
    kv = Cv.copy()
    for l in range(L):
        kt[:, l] = kv
        kv = kv @ A
    # scan matrices
    Abar = np.linalg.matrix_power(A, L)
    ups, downs = [], []
    M = Abar
    for _ in range(N_LEV):
        P4 = [np.linalg.matrix_power(M, p) for p in range(4)]
        # out rows 0:n = E_out; rows b*n:(b+1)*n = Sloc[b] for b=1..3
        up = np.zeros((4 * n, 4 * n))
        for k in range(4):
            up[0:n, k * n:(k + 1) * n] = P4[3 - k]
        for b in range(1, 4):
            for k in range(b):
                up[b * n:(b + 1) * n, k * n:(k + 1) * n] = P4[b - 1 - k]
        down = np.zeros((4 * n, n))
        for b in range(4):
            down[b * n:(b + 1) * n] = P4[b]
        ups.append(up.T)      # lhsT layout [K=in, M=out]
        downs.append(down.T)  # lhsT layout [K=j(32), M=(b,i)(128)]
        M = np.linalg.matrix_power(M, 4)
    f32 = np.float32
    return (gt.astype(f32), ut.astype(f32), kt.astype(f32),
            np.stack(ups).astype(f32), np.stack(downs).astype(f32))


def host_matrices(params):
    """params (16, 50) -> dict of stacked per-row device matrices."""
    gts, uts, kts, upss, dnss = [], [], [], [], []
    for b in range(params.shape[0]):
        gt, ut, kt, ups, dns = _row_device_matrices(params[b])
        gts.append(gt); uts.append(ut); kts.append(kt)
        upss.append(ups); dnss.append(dns)
    return dict(gt=np.stack(gts), ut=np.stack(uts), kt=np.stack(kts),
                scan_up=np.stack(upss), scan_down=np.stack(dnss))


# ---------------------------------------------------------------------------
# device program
# ---------------------------------------------------------------------------

_PROGRAM = None


def _emit(ctx, tc, nc, aps):
    import concourse.mybir as mybir
    from concourse.masks import make_identity

    f32 = mybir.dt.float32
    f32r = mybir.dt.float32r
    audio, gt, ut, kt, sup, sdn, yout = (
        aps["audio"], aps["gt"], aps["ut"], aps["kt"],
        aps["scan_up"], aps["scan_down"], aps["y"])

    const = ctx.enter_context(tc.tile_pool(name="const", bufs=1))
    data = ctx.enter_context(tc.tile_pool(name="data", bufs=1))
    pst = ctx.enter_context(tc.tile_pool(name="pst", bufs=3, space="PSUM"))
    psy = ctx.enter_context(tc.tile_pool(name="psy", bufs=3, space="PSUM"))
    psc = ctx.enter_context(tc.tile_pool(name="psc", bufs=2, space="PSUM"))
    stage = ctx.enter_context(tc.tile_pool(name="stage", bufs=16))

    ident = const.tile([128, 128], f32, name="ident", tag="ident")
    make_identity(nc, ident[:])

    # ---- input DMAs first (audio on the sync queue, block-granular) ----
    xin = [data.tile([128, NBLK, L], f32, name=f"xin{r}", tag=f"xin{r}")
           for r in range(2)]
    qi = 0
    for blk in range(NBLK):
        for r in range(2):
            asrc = audio[r].rearrange("(b p j) -> b p j", p=128, j=L)
            for q in range(NQ):
                dq = nc.sync if qi % 2 == 0 else nc.scalar
                dq.dma_start(xin[r][:, blk, q * 128:(q + 1) * 128],
                             asrc[blk][:, q * 128:(q + 1) * 128])
                qi += 1

    # ---- constants on the scalar HWDGE queue ----
    gt_sb = [const.tile([128, NQ, L], f32r, name=f"gt{r}", tag=f"gt{r}")
             for r in range(2)]
    ut_sb = [const.tile([128, NQ, NST], f32r, name=f"ut{r}", tag=f"ut{r}")
             for r in range(2)]
    kt_sb = [const.tile([NST, L], f32r, name=f"kt{r}", tag=f"kt{r}")
             for r in range(2)]
    up_sb = [[const.tile([128, 128], f32, name=f"up{r}_{v}", tag=f"up{r}_{v}")
              for v in range(N_LEV)] for r in range(2)]
    dn_sb = [[const.tile([NST, 128], f32, name=f"dn{r}_{v}", tag=f"dn{r}_{v}")
              for v in range(N_LEV)] for r in range(2)]
    for r in range(2):
        nc.scalar.dma_start(ut_sb[r][:], ut[r].bitcast(f32r))
        nc.scalar.dma_start(kt_sb[r][:], kt[r].bitcast(f32r))
        for v in range(N_LEV):
            nc.scalar.dma_start(up_sb[r][v][:], sup[r, v])
            nc.scalar.dma_start(dn_sb[r][v][:], sdn[r, v])
    for r in range(2):
        nc.scalar.dma_start(gt_sb[r][:], gt[r].bitcast(f32r))

    # ---- transpose x into [j, c] layout (f32r rounding at the copy) ----
    xT = [data.tile([128, NQ, C], f32r, name=f"xT{r}", tag=f"xT{r}")
          for r in range(2)]
    for blk in range(NBLK):
        for r in range(2):
            for q in range(NQ):
                tp = pst.tile([128, 128], f32, name="tp", tag="pst")
                nc.tensor.transpose(tp[:], xin[r][:, blk, q * 128:(q + 1) * 128],
                                    ident[:])
                dst = xT[r][:, q, blk * 128:(blk + 1) * 128]
                if q % 2 == 0:
                    nc.vector.tensor_copy(dst, tp[:])
                else:
                    nc.scalar.copy(dst, tp[:])

    # ---- phase 1a: D = U @ x ----
    D_sb = [data.tile([NST, C], f32, name=f"D{r}", tag=f"D{r}") for r in range(2)]
    for r in range(2):
        dp = pst.tile([NST, C], f32, name="dp", tag="pst")
        for q in range(NQ):
            nc.tensor.matmul(dp[:], ut_sb[r][:, q, :],
                             xT[r][:, q, :],
                             start=(q == 0), stop=(q == NQ - 1))
        nc.vector.tensor_copy(D_sb[r][:], dp[:])

    # ---- phase 2 (scan) and phase 1b (y_zs) interleaved: one scan step,
    #      then one y-block, so the in-order PE stream never stalls long ----
    E = [D_sb[r][:] for r in range(2)]
    upalls = [[], []]
    Sin = [None, None]
    S_sb = [None, None]

    def up_step(r, lev):
        n_in = SCAN_SIZES[lev]
        n_g = n_in // 4
        rhs4 = data.tile([128, n_g], f32, name=f"rhs4_{r}_{lev}", tag=f"rhs4_{r}_{lev}")
        Eg = E[r].rearrange("p (g b) -> p g b", b=4)
        for b in range(4):
            eng = nc.vector if b % 2 == 0 else nc.gpsimd
            eng.tensor_copy(rhs4[b * NST:(b + 1) * NST, :], Eg[:, :, b])
        upo = pst.tile([128, n_g], f32, name="upo", tag="pst")
        nc.tensor.matmul(upo[:], up_sb[r][lev][:], rhs4[:], start=True, stop=True)
        # rows 0:32 = E_out, rows 32:128 = Sloc[b=1..3]
        upall = data.tile([128, n_g], f32, name=f"upall_{r}_{lev}", tag=f"upall_{r}_{lev}")
        if r == 0:
            nc.vector.tensor_copy(upall[:], upo[:])
        else:
            nc.scalar.copy(upall[:], upo[:])
        upalls[r].append(upall)
        E[r] = upall[:NST, :]

    def top_step(r):
        Sin_t = data.tile([NST, 2], f32, name=f"sintop_{r}", tag=f"sintop_{r}")
        nc.any.memset(Sin_t[:, 0:1], 0.0)
        nc.vector.tensor_copy(Sin_t[:, 1:2], E[r][:, 0:1])
        Sin[r] = Sin_t[:]

    def down_step(r, lev):
        n_g = SCAN_SIZES[lev] // 4
        prop = pst.tile([128, n_g], f32, name="prop", tag="pst")
        nc.tensor.matmul(prop[:], dn_sb[r][lev][:], Sin[r], start=True, stop=True)
        full = data.tile([128, n_g], f32, name=f"full_{r}_{lev}", tag=f"full_{r}_{lev}")
        nc.scalar.copy(full[:NST, :], prop[:NST, :])
        nc.vector.tensor_tensor(full[NST:2 * NST, :], prop[NST:2 * NST, :],
                                upalls[r][lev][NST:2 * NST, :],
                                op=mybir.AluOpType.add)
        nc.vector.tensor_tensor(full[2 * NST:, :], prop[2 * NST:, :],
                                upalls[r][lev][2 * NST:, :],
                                op=mybir.AluOpType.add)
        sdt = f32r if lev == 0 else f32
        Snx = data.tile([NST, SCAN_SIZES[lev]], sdt, name=f"snx_{r}_{lev}", tag=f"snx_{r}_{lev}")
        Sg = Snx[:].rearrange("p (g b) -> p g b", b=4)
        for b in range(4):
            eng = nc.vector if b % 2 == 0 else nc.gpsimd
            eng.tensor_copy(Sg[:, :, b], full[b * NST:(b + 1) * NST, :])
        Sin[r] = Snx[:]
        if lev == 0:
            S_sb[r] = Snx[:]

    ystages = {}

    def y_block(r, blk):
        cs = slice(blk * 128, (blk + 1) * 128)
        yp = psy.tile([128, L], f32, name="yp", tag="psy")
        for q in range(NQ):
            nc.tensor.matmul(yp[:], xT[r][:, q, cs], gt_sb[r][:, q, :],
                             start=(q == 0), stop=(q == NQ - 1))
        ystage = stage.tile([128, L], f32, name="ystage", tag="ystage")
        if blk % 2 == 0:
            nc.vector.tensor_copy(ystage[:], yp[:])
        else:
            nc.scalar.copy(ystage[:], yp[:])
        ystages[(r, blk)] = ystage

    scan_steps = []
    for lev in range(N_LEV):
        for r in range(2):
            scan_steps.append(lambda r=r, lev=lev: up_step(r, lev))
    for r in range(2):
        scan_steps.append(lambda r=r: top_step(r))
    for lev in range(N_LEV - 1, -1, -1):
        for r in range(2):
            scan_steps.append(lambda r=r, lev=lev: down_step(r, lev))

    y_list = [(r, blk) for blk in range(NBLK) for r in range(2)]
    yi = 0
    for si, step in enumerate(scan_steps):
        step()
        if yi < len(y_list):
            y_block(*y_list[yi]); yi += 1
    while yi < len(y_list):
        y_block(*y_list[yi]); yi += 1

    # ---- phase 3: corrections, add, DMA out ----
    for blk in range(NBLK):
        for r in range(2):
            cs = slice(blk * 128, (blk + 1) * 128)
            ydst = yout[r].rearrange("(b p l) -> b p l", p=128, l=L)
            cp = psc.tile([128, L], f32, name="cp", tag="psc")
            nc.tensor.matmul(cp[:], S_sb[r][:, cs], kt_sb[r][:],
                             start=True, stop=True)
            ystage = ystages[(r, blk)]
            nc.vector.tensor_tensor(ystage[:], ystage[:], cp[:],
                                    op=mybir.AluOpType.add)
            dq = nc.sync if blk % 2 == 0 else nc.scalar
            dq.dma_start(ydst[blk], ystage[:])


def build_program():
    from contextlib import ExitStack

    import concourse.mybir as mybir
    import concourse.tile as tile
    from concourse import bacc

    nc = bacc.Bacc("TRN2", target_bir_lowering=False, debug=False,
                   num_devices=N_CORES)
    f32 = mybir.dt.float32
    aps = dict(
        audio=nc.dram_tensor("audio", [2, T], f32, kind="ExternalInput").ap(),
        gt=nc.dram_tensor("gt", [2, 128, NQ, L], f32, kind="ExternalInput").ap(),
        ut=nc.dram_tensor("ut", [2, 128, NQ, NST], f32, kind="ExternalInput").ap(),
        kt=nc.dram_tensor("kt", [2, NST, L], f32, kind="ExternalInput").ap(),
        scan_up=nc.dram_tensor("scan_up", [2, N_LEV, 128, 128], f32,
                               kind="ExternalInput").ap(),
        scan_down=nc.dram_tensor("scan_down", [2, N_LEV, NST, 128], f32,
                                 kind="ExternalInput").ap(),
        y=nc.dram_tensor("y", [2, T], f32, kind="ExternalOutput").ap(),
    )
    with tile.TileContext(nc) as tc:
        with ExitStack() as ctx:
            _emit(ctx, tc, nc, aps)
    nc.compile()
    return nc


def _get_program():
    global _PROGRAM
    if _PROGRAM is None:
        _PROGRAM = build_program()
    return _PROGRAM


def make_in_maps(audio, params):
    audio = np.ascontiguousarray(np.asarray(audio, np.float32))
    mats = host_matrices(np.asarray(params, np.float64))
    in_maps = []
    for i in range(N_CORES):
        s = slice(2 * i, 2 * i + 2)
        in_maps.append({
            "audio": audio[s],
            "gt": mats["gt"][s],
            "ut": mats["ut"][s],
            "kt": mats["kt"][s],
            "scan_up": mats["scan_up"][s],
            "scan_down": mats["scan_down"][s],
        })
    return in_maps


def kernel(audio, params):
    from concourse.bass_utils import run_bass_kernel_spmd

    nc = _get_program()
    in_maps = make_in_maps(audio, params)
    res = run_bass_kernel_spmd(nc, in_maps, list(range(N_CORES)))
    return np.concatenate([res.results[i]["y"] for i in range(N_CORES)],
                          axis=0).astype(np.float32)


# revision 24
# speedup vs baseline: 1.1147x; 1.0780x over previous
"""Trainium2 Bass kernel for nn_DifferentiableBiquadChain.

16 cascaded biquads over (16, 262144) audio. The whole cascade is one LTI
system with a 32-dim state; we decompose each batch row's sequence into 2048
chunks of 128 samples and compute:

  phase 1a: per-chunk end-state contribution  D_c = U @ x_c        (TensorE)
  phase 2:  boundary-state scan S_{c+1} = A^128 S_c + D_c, done as a
            radix-4 hierarchical matmul scan over precomputed A-powers
  phase 1b: per-chunk zero-state response  y_zs = Toeplitz(g) @ x_c (TensorE)
  phase 3:  correction y_c = y_zs + Kmat @ S_c, PSUM-accumulated

All matrices (impulse response g, U, Kmat, scan-level A-power blocks) are
precomputed on host in float64 from `params`; the device does only fp32
matmuls + copies. Data-parallel: 2 batch rows per core on 8 cores.
"""
import math

import numpy as np

FS = 96000.0
N_BIQUADS = 16
HPF_FREQ_RANGE = (20.0, 500.0)
LPF_FREQ_RANGE = (5000.0, 20000.0)
SHELF_FREQ_RANGE = (50.0, 16000.0)
PEAK_FREQ_RANGE = (100.0, 15000.0)
Q_RANGE = (0.5, 16.0)
GAIN_RANGE = (-24.0, 24.0)
BROADBAND_RANGE = (-60.0, 0.0)

T = 262144
L = 512          # chunk length
NQ = L // 128    # K-quarters per chunk
C = T // L       # 512 chunks per row
NBLK = C // 128  # 4 chunk-blocks per row
NST = 2 * N_BIQUADS   # 32 state dims
ROWS_PER_CORE = 2
N_CORES = 8
SCAN_SIZES = [512, 128, 32, 8]  # radix-4 upsweep input sizes
N_LEV = len(SCAN_SIZES)


# ---------------------------------------------------------------------------
# host-side math (float64)
# ---------------------------------------------------------------------------

def _denorm_log(norm, lo, hi):
    return np.exp(math.log(lo) + norm * (math.log(hi) - math.log(lo)))


def _coef_highpass(fc, Q):
    w0 = 2.0 * math.pi * fc / FS
    alpha = np.sin(w0) / (2.0 * Q)
    c = np.cos(w0)
    b0 = (1 + c) / 2; b1 = -(1 + c); b2 = (1 + c) / 2
    a0 = 1 + alpha; a1 = -2 * c; a2 = 1 - alpha
    return b0 / a0, b1 / a0, b2 / a0, a1 / a0, a2 / a0


def _coef_lowpass(fc, Q):
    w0 = 2.0 * math.pi * fc / FS
    alpha = np.sin(w0) / (2.0 * Q)
    c = np.cos(w0)
    b0 = (1 - c) / 2; b1 = 1 - c; b2 = (1 - c) / 2
    a0 = 1 + alpha; a1 = -2 * c; a2 = 1 - alpha
    return b0 / a0, b1 / a0, b2 / a0, a1 / a0, a2 / a0


def _coef_lowshelf(fc, gain_db, Q):
    A = 10.0 ** (gain_db / 40.0)
    w0 = 2.0 * math.pi * fc / FS
    alpha = np.sin(w0) / (2.0 * Q)
    c = np.cos(w0)
    sA = np.sqrt(A)
    b0 = A * (A + 1 - (A - 1) * c + 2 * sA * alpha)
    b1 = 2 * A * (A - 1 - (A + 1) * c)
    b2 = A * (A + 1 - (A - 1) * c - 2 * sA * alpha)
    a0 = A + 1 + (A - 1) * c + 2 * sA * alpha
    a1 = -2 * (A - 1 + (A + 1) * c)
    a2 = A + 1 + (A - 1) * c - 2 * sA * alpha
    return b0 / a0, b1 / a0, b2 / a0, a1 / a0, a2 / a0


def _coef_highshelf(fc, gain_db, Q):
    A = 10.0 ** (gain_db / 40.0)
    w0 = 2.0 * math.pi * fc / FS
    alpha = np.sin(w0) / (2.0 * Q)
    c = np.cos(w0)
    sA = np.sqrt(A)
    b0 = A * (A + 1 + (A - 1) * c + 2 * sA * alpha)
    b1 = -2 * A * (A - 1 + (A + 1) * c)
    b2 = A * (A + 1 + (A - 1) * c - 2 * sA * alpha)
    a0 = A + 1 - (A - 1) * c + 2 * sA * alpha
    a1 = 2 * (A - 1 - (A + 1) * c)
    a2 = A + 1 - (A - 1) * c - 2 * sA * alpha
    return b0 / a0, b1 / a0, b2 / a0, a1 / a0, a2 / a0


def _coef_peak(fc, gain_db, Q):
    A = 10.0 ** (gain_db / 40.0)
    w0 = 2.0 * math.pi * fc / FS
    alpha = np.sin(w0) / (2.0 * Q)
    c = np.cos(w0)
    b0 = 1 + alpha * A; b1 = -2 * c; b2 = 1 - alpha * A
    a0 = 1 + alpha / A; a1 = -2 * c; a2 = 1 - alpha / A
    return b0 / a0, b1 / a0, b2 / a0, a1 / a0, a2 / a0


def _row_coeffs(p_row):
    bp = p_row[: N_BIQUADS * 3].reshape(N_BIQUADS, 3)
    bb_lo, bb_hi = BROADBAND_RANGE
    in_gain = 10.0 ** ((bb_lo + p_row[-2] * (bb_hi - bb_lo)) / 20.0)
    out_gain = 10.0 ** ((bb_lo + p_row[-1] * (bb_hi - bb_lo)) / 20.0)
    coefs = []
    for i in range(N_BIQUADS):
        fn, gn, qn = bp[i, 0], bp[i, 1], bp[i, 2]
        Q = _denorm_log(qn, *Q_RANGE)
        gain = GAIN_RANGE[0] + gn * (GAIN_RANGE[1] - GAIN_RANGE[0])
        if i == 0:
            cf = _coef_highpass(_denorm_log(fn, *HPF_FREQ_RANGE), Q)
        elif i == 15:
            cf = _coef_lowpass(_denorm_log(fn, *LPF_FREQ_RANGE), Q)
        elif i == 1:
            cf = _coef_lowshelf(_denorm_log(fn, *SHELF_FREQ_RANGE), gain, Q)
        elif i == 14:
            cf = _coef_highshelf(_denorm_log(fn, *SHELF_FREQ_RANGE), gain, Q)
        else:
            cf = _coef_peak(_denorm_log(fn, *PEAK_FREQ_RANGE), gain, Q)
        coefs.append(tuple(float(v) for v in cf))
    return coefs, float(in_gain), float(out_gain)


def _cascade_statespace(coefs, in_gain, out_gain):
    """Full-cascade state space (A, B, C, Dff), DF2-transposed per biquad."""
    n = NST
    A = np.zeros((n, n))
    B = np.zeros(n)
    d_u = in_gain
    c_u = np.zeros(n)
    for k, (b0, b1, b2, a1, a2) in enumerate(coefs):
        e1 = np.zeros(n); e1[2 * k] = 1.0
        e2 = np.zeros(n); e2[2 * k + 1] = 1.0
        d_y = b0 * d_u
        c_y = b0 * c_u + e1
        A[2 * k] = b1 * c_u - a1 * c_y + e2
        B[2 * k] = b1 * d_u - a1 * d_y
        A[2 * k + 1] = b2 * c_u - a2 * c_y
        B[2 * k + 1] = b2 * d_u - a2 * d_y
        d_u, c_u = d_y, c_y
    return A, B, out_gain * c_u, out_gain * d_u


def _modal_realization(A, B, Cv):
    """Similarity-transform to a real modal (2x2 block-diagonal) basis with
    input/output-balanced mode scaling. Same transfer function; states become
    well-scaled so f32r rounding in D = U@x and y_corr = K@S stays benign."""
    n = A.shape[0]
    w, V = np.linalg.eig(A)
    T = np.zeros((n, n))
    used = np.zeros(len(w), bool)
    col = 0
    for i in range(len(w)):
        if used[i]:
            continue
        li, vi = w[i], V[:, i]
        if abs(li.imag) < 1e-12:
            T[:, col] = vi.real / np.linalg.norm(vi.real)
            used[i] = True
            col += 1
            continue
        # find conjugate partner
        j = min((jj for jj in range(len(w)) if not used[jj] and jj != i),
                key=lambda jj: abs(w[jj] - np.conj(li)))
        re, im = vi.real, vi.imag
        nrm = max(np.linalg.norm(re), np.linalg.norm(im))
        T[:, col] = re / nrm
        T[:, col + 1] = im / nrm
        used[i] = used[j] = True
        col += 2
    Ti = np.linalg.inv(T)
    A2, B2, C2 = Ti @ A @ T, Ti @ B, Cv @ T
    # balance per-state input vs output weighting
    s = np.sqrt((np.abs(B2) + 1e-30) / (np.abs(C2) + 1e-30))
    s = np.clip(s, 1e-6, 1e6)
    A2 = A2 * (s[None, :] / s[:, None])
    B2 = B2 / s
    C2 = C2 * s
    return A2, B2, C2


def _row_device_matrices(p_row):
    """Float32 matrices for one batch row, laid out exactly as the device
    matmuls consume them."""
    coefs, ig, og = _row_coeffs(np.asarray(p_row, np.float64))
    A, B, Cv, Dff = _cascade_statespace(coefs, ig, og)
    A, B, Cv = _modal_realization(A, B, Cv)
    n = NST
    # impulse response g[0..L-1]
    g = np.zeros(L)
    g[0] = Dff
    v = B.copy()
    for l in range(1, L):
        g[l] = Cv @ v
        v = A @ v
    # gt[j', q, l] = g[l - 128q - j']  (rhs of the y_zs matmul, K-quarter q)
    gt = np.zeros((128, NQ, L))
    for q in range(NQ):
        for jp in range(128):
            j = 128 * q + jp
            gt[jp, q, j:] = g[: L - j]
    # ut[q, j', i] = U[i, 128q + j'] = (A^{L-1-j} B)[i]  (lhsT of the D matmul)
    ut_full = np.zeros((L, n))
    w = B.copy()
    for j in range(L - 1, -1, -1):
        ut_full[j] = w
        w = A @ w
    ut = ut_full.reshape(NQ, 128, n).transpose(1, 0, 2)  # [j', q, i]
    # kt[i, l] = Kmat[l, i] = (C A^l)[i]  (rhs of the correction matmul)
    kt = np.zeros((n, L))
    kv = Cv.copy()
    for l in range(L):
        kt[:, l] = kv
        kv = kv @ A
    # scan matrices
    Abar = np.linalg.matrix_power(A, L)
    ups, downs = [], []
    M = Abar
    for _ in range(N_LEV):
        P4 = [np.linalg.matrix_power(M, p) for p in range(4)]
        # out rows 0:n = E_out; rows b*n:(b+1)*n = Sloc[b] for b=1..3
        up = np.zeros((4 * n, 4 * n))
        for k in range(4):
            up[0:n, k * n:(k + 1) * n] = P4[3 - k]
        for b in range(1, 4):
            for k in range(b):
                up[b * n:(b + 1) * n, k * n:(k + 1) * n] = P4[b - 1 - k]
        down = np.zeros((4 * n, n))
        for b in range(4):
            down[b * n:(b + 1) * n] = P4[b]
        ups.append(up.T)      # lhsT layout [K=in, M=out]
        downs.append(down.T)  # lhsT layout [K=j(32), M=(b,i)(128)]
        M = np.linalg.matrix_power(M, 4)
    f32 = np.float32
    return (gt.astype(f32), ut.astype(f32), kt.astype(f32),
            np.stack(ups).astype(f32), np.stack(downs).astype(f32))


def host_matrices(params):
    """params (16, 50) -> dict of stacked per-row device matrices."""
    gts, uts, kts, upss, dnss = [], [], [], [], []
    for b in range(params.shape[0]):
        gt, ut, kt, ups, dns = _row_device_matrices(params[b])
        gts.append(gt); uts.append(ut); kts.append(kt)
        upss.append(ups); dnss.append(dns)
    return dict(gt=np.stack(gts), ut=np.stack(uts), kt=np.stack(kts),
                scan_up=np.stack(upss), scan_down=np.stack(dnss))


# ---------------------------------------------------------------------------
# device program
# ---------------------------------------------------------------------------

_PROGRAM = None


def _emit(ctx, tc, nc, aps):
    import concourse.mybir as mybir
    from concourse.masks import make_identity

    f32 = mybir.dt.float32
    f32r = mybir.dt.float32r
    audio, gt, ut, kt, sup, sdn, yout = (
        aps["audio"], aps["gt"], aps["ut"], aps["kt"],
        aps["scan_up"], aps["scan_down"], aps["y"])

    const = ctx.enter_context(tc.tile_pool(name="const", bufs=1))
    data = ctx.enter_context(tc.tile_pool(name="data", bufs=1))
    pst = ctx.enter_context(tc.tile_pool(name="pst", bufs=3, space="PSUM"))
    psy = ctx.enter_context(tc.tile_pool(name="psy", bufs=3, space="PSUM"))
    psc = ctx.enter_context(tc.tile_pool(name="psc", bufs=2, space="PSUM"))
    stage = ctx.enter_context(tc.tile_pool(name="stage", bufs=16))

    ident = const.tile([128, 128], f32, name="ident", tag="ident")
    make_identity(nc, ident[:])

    # ---- input DMAs first (audio on the sync queue, block-granular) ----
    xin = [data.tile([128, NBLK, L], f32, name=f"xin{r}", tag=f"xin{r}")
           for r in range(2)]
    qi = 0
    for blk in range(NBLK):
        for r in range(2):
            asrc = audio[r].rearrange("(b p j) -> b p j", p=128, j=L)
            dq = nc.sync if qi % 2 == 0 else nc.scalar
            dq.dma_start(xin[r][:, blk, :], asrc[blk])
            qi += 1

    # ---- constants on the scalar HWDGE queue ----
    gt_sb = [const.tile([128, NQ, L], f32r, name=f"gt{r}", tag=f"gt{r}")
             for r in range(2)]
    ut_sb = [const.tile([128, NQ, NST], f32r, name=f"ut{r}", tag=f"ut{r}")
             for r in range(2)]
    kt_sb = [const.tile([NST, L], f32r, name=f"kt{r}", tag=f"kt{r}")
             for r in range(2)]
    up_sb = [[const.tile([128, 128], f32, name=f"up{r}_{v}", tag=f"up{r}_{v}")
              for v in range(N_LEV)] for r in range(2)]
    dn_sb = [[const.tile([NST, 128], f32, name=f"dn{r}_{v}", tag=f"dn{r}_{v}")
              for v in range(N_LEV)] for r in range(2)]
    for r in range(2):
        nc.scalar.dma_start(ut_sb[r][:], ut[r].bitcast(f32r))
        nc.scalar.dma_start(kt_sb[r][:], kt[r].bitcast(f32r))
        for v in range(N_LEV):
            nc.scalar.dma_start(up_sb[r][v][:], sup[r, v])
            nc.scalar.dma_start(dn_sb[r][v][:], sdn[r, v])
    for r in range(2):
        nc.scalar.dma_start(gt_sb[r][:], gt[r].bitcast(f32r))

    # ---- transpose x into [j, c] layout (f32r rounding at the copy) ----
    xT = [data.tile([128, NQ, C], f32r, name=f"xT{r}", tag=f"xT{r}")
          for r in range(2)]
    for blk in range(NBLK):
        for r in range(2):
            for q in range(NQ):
                tp = pst.tile([128, 128], f32, name="tp", tag="pst")
                nc.tensor.transpose(tp[:], xin[r][:, blk, q * 128:(q + 1) * 128],
                                    ident[:])
                dst = xT[r][:, q, blk * 128:(blk + 1) * 128]
                if q % 2 == 0:
                    nc.vector.tensor_copy(dst, tp[:])
                else:
                    nc.scalar.copy(dst, tp[:])

    # ---- phase 1a: D = U @ x ----
    D_sb = [data.tile([NST, C], f32, name=f"D{r}", tag=f"D{r}") for r in range(2)]
    for r in range(2):
        dp = pst.tile([NST, C], f32, name="dp", tag="pst")
        for q in range(NQ):
            nc.tensor.matmul(dp[:], ut_sb[r][:, q, :],
                             xT[r][:, q, :],
                             start=(q == 0), stop=(q == NQ - 1))
        nc.vector.tensor_copy(D_sb[r][:], dp[:])

    # ---- phase 2 (scan) and phase 1b (y_zs) interleaved: one scan step,
    #      then one y-block, so the in-order PE stream never stalls long ----
    E = [D_sb[r][:] for r in range(2)]
    upalls = [[], []]
    Sin = [None, None]
    S_sb = [None, None]

    def up_step(r, lev):
        n_in = SCAN_SIZES[lev]
        n_g = n_in // 4
        rhs4 = data.tile([128, n_g], f32, name=f"rhs4_{r}_{lev}", tag=f"rhs4_{r}_{lev}")
        Eg = E[r].rearrange("p (g b) -> p g b", b=4)
        for b in range(4):
            eng = nc.vector if b % 2 == 0 else nc.gpsimd
            eng.tensor_copy(rhs4[b * NST:(b + 1) * NST, :], Eg[:, :, b])
        upo = pst.tile([128, n_g], f32, name="upo", tag="pst")
        nc.tensor.matmul(upo[:], up_sb[r][lev][:], rhs4[:], start=True, stop=True)
        # rows 0:32 = E_out, rows 32:128 = Sloc[b=1..3]
        upall = data.tile([128, n_g], f32, name=f"upall_{r}_{lev}", tag=f"upall_{r}_{lev}")
        if r == 0:
            nc.vector.tensor_copy(upall[:], upo[:])
        else:
            nc.scalar.copy(upall[:], upo[:])
        upalls[r].append(upall)
        E[r] = upall[:NST, :]

    def top_step(r):
        Sin_t = data.tile([NST, 2], f32, name=f"sintop_{r}", tag=f"sintop_{r}")
        nc.any.memset(Sin_t[:, 0:1], 0.0)
        nc.vector.tensor_copy(Sin_t[:, 1:2], E[r][:, 0:1])
        Sin[r] = Sin_t[:]

    def down_step(r, lev):
        n_g = SCAN_SIZES[lev] // 4
        prop = pst.tile([128, n_g], f32, name="prop", tag="pst")
        nc.tensor.matmul(prop[:], dn_sb[r][lev][:], Sin[r], start=True, stop=True)
        full = data.tile([128, n_g], f32, name=f"full_{r}_{lev}", tag=f"full_{r}_{lev}")
        nc.scalar.copy(full[:NST, :], prop[:NST, :])
        nc.vector.tensor_tensor(full[NST:2 * NST, :], prop[NST:2 * NST, :],
                                upalls[r][lev][NST:2 * NST, :],
                                op=mybir.AluOpType.add)
        nc.vector.tensor_tensor(full[2 * NST:, :], prop[2 * NST:, :],
                                upalls[r][lev][2 * NST:, :],
                                op=mybir.AluOpType.add)
        sdt = f32r if lev == 0 else f32
        Snx = data.tile([NST, SCAN_SIZES[lev]], sdt, name=f"snx_{r}_{lev}", tag=f"snx_{r}_{lev}")
        Sg = Snx[:].rearrange("p (g b) -> p g b", b=4)
        for b in range(4):
            eng = nc.vector if b % 2 == 0 else nc.gpsimd
            eng.tensor_copy(Sg[:, :, b], full[b * NST:(b + 1) * NST, :])
        Sin[r] = Snx[:]
        if lev == 0:
            S_sb[r] = Snx[:]

    ystages = {}

    def y_block(r, blk):
        cs = slice(blk * 128, (blk + 1) * 128)
        yp = psy.tile([128, L], f32, name="yp", tag="psy")
        for q in range(NQ):
            nc.tensor.matmul(yp[:], xT[r][:, q, cs], gt_sb[r][:, q, :],
                             start=(q == 0), stop=(q == NQ - 1))
        ystage = stage.tile([128, L], f32, name="ystage", tag="ystage")
        if blk % 2 == 0:
            nc.vector.tensor_copy(ystage[:], yp[:])
        else:
            nc.scalar.copy(ystage[:], yp[:])
        ystages[(r, blk)] = ystage

    scan_steps = []
    for lev in range(N_LEV):
        for r in range(2):
            scan_steps.append(lambda r=r, lev=lev: up_step(r, lev))
    for r in range(2):
        scan_steps.append(lambda r=r: top_step(r))
    for lev in range(N_LEV - 1, -1, -1):
        for r in range(2):
            scan_steps.append(lambda r=r, lev=lev: down_step(r, lev))

    y_list = [(r, blk) for blk in range(NBLK) for r in range(2)]
    yi = 0
    for si, step in enumerate(scan_steps):
        step()
        if yi < len(y_list):
            y_block(*y_list[yi]); yi += 1
    while yi < len(y_list):
        y_block(*y_list[yi]); yi += 1

    # ---- phase 3: corrections, add, DMA out ----
    for blk in range(NBLK):
        for r in range(2):
            cs = slice(blk * 128, (blk + 1) * 128)
            ydst = yout[r].rearrange("(b p l) -> b p l", p=128, l=L)
            cp = psc.tile([128, L], f32, name="cp", tag="psc")
            nc.tensor.matmul(cp[:], S_sb[r][:, cs], kt_sb[r][:],
                             start=True, stop=True)
            ystage = ystages[(r, blk)]
            nc.vector.tensor_tensor(ystage[:], ystage[:], cp[:],
                                    op=mybir.AluOpType.add)
            dq = nc.sync if blk % 2 == 0 else nc.scalar
            dq.dma_start(ydst[blk], ystage[:])


def build_program():
    from contextlib import ExitStack

    import concourse.mybir as mybir
    import concourse.tile as tile
    from concourse import bacc

    nc = bacc.Bacc("TRN2", target_bir_lowering=False, debug=False,
                   num_devices=N_CORES)
    f32 = mybir.dt.float32
    aps = dict(
        audio=nc.dram_tensor("audio", [2, T], f32, kind="ExternalInput").ap(),
        gt=nc.dram_tensor("gt", [2, 128, NQ, L], f32, kind="ExternalInput").ap(),
        ut=nc.dram_tensor("ut", [2, 128, NQ, NST], f32, kind="ExternalInput").ap(),
        kt=nc.dram_tensor("kt", [2, NST, L], f32, kind="ExternalInput").ap(),
        scan_up=nc.dram_tensor("scan_up", [2, N_LEV, 128, 128], f32,
                               kind="ExternalInput").ap(),
        scan_down=nc.dram_tensor("scan_down", [2, N_LEV, NST, 128], f32,
                                 kind="ExternalInput").ap(),
        y=nc.dram_tensor("y", [2, T], f32, kind="ExternalOutput").ap(),
    )
    with tile.TileContext(nc) as tc:
        with ExitStack() as ctx:
            _emit(ctx, tc, nc, aps)
    nc.compile()
    return nc


def _get_program():
    global _PROGRAM
    if _PROGRAM is None:
        _PROGRAM = build_program()
    return _PROGRAM


def make_in_maps(audio, params):
    audio = np.ascontiguousarray(np.asarray(audio, np.float32))
    mats = host_matrices(np.asarray(params, np.float64))
    in_maps = []
    for i in range(N_CORES):
        s = slice(2 * i, 2 * i + 2)
        in_maps.append({
            "audio": audio[s],
            "gt": mats["gt"][s],
            "ut": mats["ut"][s],
            "kt": mats["kt"][s],
            "scan_up": mats["scan_up"][s],
            "scan_down": mats["scan_down"][s],
        })
    return in_maps


def kernel(audio, params):
    from concourse.bass_utils import run_bass_kernel_spmd

    nc = _get_program()
    in_maps = make_in_maps(audio, params)
    res = run_bass_kernel_spmd(nc, in_maps, list(range(N_CORES)))
    return np.concatenate([res.results[i]["y"] for i in range(N_CORES)],
                          axis=0).astype(np.float32)


# revision 25
# speedup vs baseline: 1.3714x; 1.2303x over previous
"""Trainium2 Bass kernel for nn_DifferentiableBiquadChain.

16 cascaded biquads over (16, 262144) audio. The whole cascade is one LTI
system with a 32-dim state; we decompose each batch row's sequence into 2048
chunks of 128 samples and compute:

  phase 1a: per-chunk end-state contribution  D_c = U @ x_c        (TensorE)
  phase 2:  boundary-state scan S_{c+1} = A^128 S_c + D_c, done as a
            radix-4 hierarchical matmul scan over precomputed A-powers
  phase 1b: per-chunk zero-state response  y_zs = Toeplitz(g) @ x_c (TensorE)
  phase 3:  correction y_c = y_zs + Kmat @ S_c, PSUM-accumulated

All matrices (impulse response g, U, Kmat, scan-level A-power blocks) are
precomputed on host in float64 from `params`; the device does only fp32
matmuls + copies. Data-parallel: 2 batch rows per core on 8 cores.
"""
import math

import numpy as np

FS = 96000.0
N_BIQUADS = 16
HPF_FREQ_RANGE = (20.0, 500.0)
LPF_FREQ_RANGE = (5000.0, 20000.0)
SHELF_FREQ_RANGE = (50.0, 16000.0)
PEAK_FREQ_RANGE = (100.0, 15000.0)
Q_RANGE = (0.5, 16.0)
GAIN_RANGE = (-24.0, 24.0)
BROADBAND_RANGE = (-60.0, 0.0)

T = 262144
L = 512          # chunk length
NQ = L // 128    # K-quarters per chunk
C = T // L       # 512 chunks per row
NBLK = C // 128  # 4 chunk-blocks per row
NST = 2 * N_BIQUADS   # 32 state dims
ROWS_PER_CORE = 2
N_CORES = 8
SCAN_SIZES = [512, 128, 32, 8]  # radix-4 upsweep input sizes
N_LEV = len(SCAN_SIZES)


# ---------------------------------------------------------------------------
# host-side math (float64)
# ---------------------------------------------------------------------------

def _denorm_log(norm, lo, hi):
    return np.exp(math.log(lo) + norm * (math.log(hi) - math.log(lo)))


def _coef_highpass(fc, Q):
    w0 = 2.0 * math.pi * fc / FS
    alpha = np.sin(w0) / (2.0 * Q)
    c = np.cos(w0)
    b0 = (1 + c) / 2; b1 = -(1 + c); b2 = (1 + c) / 2
    a0 = 1 + alpha; a1 = -2 * c; a2 = 1 - alpha
    return b0 / a0, b1 / a0, b2 / a0, a1 / a0, a2 / a0


def _coef_lowpass(fc, Q):
    w0 = 2.0 * math.pi * fc / FS
    alpha = np.sin(w0) / (2.0 * Q)
    c = np.cos(w0)
    b0 = (1 - c) / 2; b1 = 1 - c; b2 = (1 - c) / 2
    a0 = 1 + alpha; a1 = -2 * c; a2 = 1 - alpha
    return b0 / a0, b1 / a0, b2 / a0, a1 / a0, a2 / a0


def _coef_lowshelf(fc, gain_db, Q):
    A = 10.0 ** (gain_db / 40.0)
    w0 = 2.0 * math.pi * fc / FS
    alpha = np.sin(w0) / (2.0 * Q)
    c = np.cos(w0)
    sA = np.sqrt(A)
    b0 = A * (A + 1 - (A - 1) * c + 2 * sA * alpha)
    b1 = 2 * A * (A - 1 - (A + 1) * c)
    b2 = A * (A + 1 - (A - 1) * c - 2 * sA * alpha)
    a0 = A + 1 + (A - 1) * c + 2 * sA * alpha
    a1 = -2 * (A - 1 + (A + 1) * c)
    a2 = A + 1 + (A - 1) * c - 2 * sA * alpha
    return b0 / a0, b1 / a0, b2 / a0, a1 / a0, a2 / a0


def _coef_highshelf(fc, gain_db, Q):
    A = 10.0 ** (gain_db / 40.0)
    w0 = 2.0 * math.pi * fc / FS
    alpha = np.sin(w0) / (2.0 * Q)
    c = np.cos(w0)
    sA = np.sqrt(A)
    b0 = A * (A + 1 + (A - 1) * c + 2 * sA * alpha)
    b1 = -2 * A * (A - 1 + (A + 1) * c)
    b2 = A * (A + 1 + (A - 1) * c - 2 * sA * alpha)
    a0 = A + 1 - (A - 1) * c + 2 * sA * alpha
    a1 = 2 * (A - 1 - (A + 1) * c)
    a2 = A + 1 - (A - 1) * c - 2 * sA * alpha
    return b0 / a0, b1 / a0, b2 / a0, a1 / a0, a2 / a0


def _coef_peak(fc, gain_db, Q):
    A = 10.0 ** (gain_db / 40.0)
    w0 = 2.0 * math.pi * fc / FS
    alpha = np.sin(w0) / (2.0 * Q)
    c = np.cos(w0)
    b0 = 1 + alpha * A; b1 = -2 * c; b2 = 1 - alpha * A
    a0 = 1 + alpha / A; a1 = -2 * c; a2 = 1 - alpha / A
    return b0 / a0, b1 / a0, b2 / a0, a1 / a0, a2 / a0


def _row_coeffs(p_row):
    bp = p_row[: N_BIQUADS * 3].reshape(N_BIQUADS, 3)
    bb_lo, bb_hi = BROADBAND_RANGE
    in_gain = 10.0 ** ((bb_lo + p_row[-2] * (bb_hi - bb_lo)) / 20.0)
    out_gain = 10.0 ** ((bb_lo + p_row[-1] * (bb_hi - bb_lo)) / 20.0)
    coefs = []
    for i in range(N_BIQUADS):
        fn, gn, qn = bp[i, 0], bp[i, 1], bp[i, 2]
        Q = _denorm_log(qn, *Q_RANGE)
        gain = GAIN_RANGE[0] + gn * (GAIN_RANGE[1] - GAIN_RANGE[0])
        if i == 0:
            cf = _coef_highpass(_denorm_log(fn, *HPF_FREQ_RANGE), Q)
        elif i == 15:
            cf = _coef_lowpass(_denorm_log(fn, *LPF_FREQ_RANGE), Q)
        elif i == 1:
            cf = _coef_lowshelf(_denorm_log(fn, *SHELF_FREQ_RANGE), gain, Q)
        elif i == 14:
            cf = _coef_highshelf(_denorm_log(fn, *SHELF_FREQ_RANGE), gain, Q)
        else:
            cf = _coef_peak(_denorm_log(fn, *PEAK_FREQ_RANGE), gain, Q)
        coefs.append(tuple(float(v) for v in cf))
    return coefs, float(in_gain), float(out_gain)


def _cascade_statespace(coefs, in_gain, out_gain):
    """Full-cascade state space (A, B, C, Dff), DF2-transposed per biquad."""
    n = NST
    A = np.zeros((n, n))
    B = np.zeros(n)
    d_u = in_gain
    c_u = np.zeros(n)
    for k, (b0, b1, b2, a1, a2) in enumerate(coefs):
        e1 = np.zeros(n); e1[2 * k] = 1.0
        e2 = np.zeros(n); e2[2 * k + 1] = 1.0
        d_y = b0 * d_u
        c_y = b0 * c_u + e1
        A[2 * k] = b1 * c_u - a1 * c_y + e2
        B[2 * k] = b1 * d_u - a1 * d_y
        A[2 * k + 1] = b2 * c_u - a2 * c_y
        B[2 * k + 1] = b2 * d_u - a2 * d_y
        d_u, c_u = d_y, c_y
    return A, B, out_gain * c_u, out_gain * d_u


def _modal_realization(A, B, Cv):
    """Similarity-transform to a real modal (2x2 block-diagonal) basis with
    input/output-balanced mode scaling. Same transfer function; states become
    well-scaled so f32r rounding in D = U@x and y_corr = K@S stays benign."""
    n = A.shape[0]
    w, V = np.linalg.eig(A)
    T = np.zeros((n, n))
    used = np.zeros(len(w), bool)
    col = 0
    for i in range(len(w)):
        if used[i]:
            continue
        li, vi = w[i], V[:, i]
        if abs(li.imag) < 1e-12:
            T[:, col] = vi.real / np.linalg.norm(vi.real)
            used[i] = True
            col += 1
            continue
        # find conjugate partner
        j = min((jj for jj in range(len(w)) if not used[jj] and jj != i),
                key=lambda jj: abs(w[jj] - np.conj(li)))
        re, im = vi.real, vi.imag
        nrm = max(np.linalg.norm(re), np.linalg.norm(im))
        T[:, col] = re / nrm
        T[:, col + 1] = im / nrm
        used[i] = used[j] = True
        col += 2
    Ti = np.linalg.inv(T)
    A2, B2, C2 = Ti @ A @ T, Ti @ B, Cv @ T
    # balance per-state input vs output weighting
    s = np.sqrt((np.abs(B2) + 1e-30) / (np.abs(C2) + 1e-30))
    s = np.clip(s, 1e-6, 1e6)
    A2 = A2 * (s[None, :] / s[:, None])
    B2 = B2 / s
    C2 = C2 * s
    return A2, B2, C2


def _row_device_matrices(p_row):
    """Float32 matrices for one batch row, laid out exactly as the device
    matmuls consume them."""
    coefs, ig, og = _row_coeffs(np.asarray(p_row, np.float64))
    A, B, Cv, Dff = _cascade_statespace(coefs, ig, og)
    A, B, Cv = _modal_realization(A, B, Cv)
    n = NST
    # impulse response g[0..L-1]
    g = np.zeros(L)
    g[0] = Dff
    v = B.copy()
    for l in range(1, L):
        g[l] = Cv @ v
        v = A @ v
    # gt[j', q, l] = g[l - 128q - j']  (rhs of the y_zs matmul, K-quarter q)
    gt = np.zeros((128, NQ, L))
    for q in range(NQ):
        for jp in range(128):
            j = 128 * q + jp
            gt[jp, q, j:] = g[: L - j]
    # ut[q, j', i] = U[i, 128q + j'] = (A^{L-1-j} B)[i]  (lhsT of the D matmul)
    ut_full = np.zeros((L, n))
    w = B.copy()
    for j in range(L - 1, -1, -1):
        ut_full[j] = w
        w = A @ w
    ut = ut_full.reshape(NQ, 128, n).transpose(1, 0, 2)  # [j', q, i]
    # kt[i, l] = Kmat[l, i] = (C A^l)[i]  (rhs of the correction matmul)
    kt = np.zeros((n, L))
    kv = Cv.copy()
    for l in range(L):
        kt[:, l] = kv
        kv = kv @ A
    # scan matrices
    Abar = np.linalg.matrix_power(A, L)
    ups, downs = [], []
    M = Abar
    for _ in range(N_LEV):
        P4 = [np.linalg.matrix_power(M, p) for p in range(4)]
        # out rows 0:n = E_out; rows b*n:(b+1)*n = Sloc[b] for b=1..3
        up = np.zeros((4 * n, 4 * n))
        for k in range(4):
            up[0:n, k * n:(k + 1) * n] = P4[3 - k]
        for b in range(1, 4):
            for k in range(b):
                up[b * n:(b + 1) * n, k * n:(k + 1) * n] = P4[b - 1 - k]
        down = np.zeros((4 * n, n))
        for b in range(4):
            down[b * n:(b + 1) * n] = P4[b]
        ups.append(up.T)      # lhsT layout [K=in, M=out]
        downs.append(down.T)  # lhsT layout [K=j(32), M=(b,i)(128)]
        M = np.linalg.matrix_power(M, 4)
    f32 = np.float32
    return (gt.astype(f32), ut.astype(f32), kt.astype(f32),
            np.stack(ups).astype(f32), np.stack(downs).astype(f32))


def host_matrices(params):
    """params (16, 50) -> dict of stacked per-row device matrices."""
    gts, uts, kts, upss, dnss = [], [], [], [], []
    for b in range(params.shape[0]):
        gt, ut, kt, ups, dns = _row_device_matrices(params[b])
        gts.append(gt); uts.append(ut); kts.append(kt)
        upss.append(ups); dnss.append(dns)
    return dict(gt=np.stack(gts), ut=np.stack(uts), kt=np.stack(kts),
                scan_up=np.stack(upss), scan_down=np.stack(dnss))


# ---------------------------------------------------------------------------
# device program
# ---------------------------------------------------------------------------

_PROGRAM = None


def _emit(ctx, tc, nc, aps):
    import concourse.mybir as mybir
    from concourse.masks import make_identity

    f32 = mybir.dt.float32
    f32r = mybir.dt.float32r
    audio, gt, ut, kt, sup, sdn, yout = (
        aps["audio"], aps["gt"], aps["ut"], aps["kt"],
        aps["scan_up"], aps["scan_down"], aps["y"])

    const = ctx.enter_context(tc.tile_pool(name="const", bufs=1))
    data = ctx.enter_context(tc.tile_pool(name="data", bufs=1))
    pst = ctx.enter_context(tc.tile_pool(name="pst", bufs=3, space="PSUM"))
    psy = ctx.enter_context(tc.tile_pool(name="psy", bufs=3, space="PSUM"))
    psc = ctx.enter_context(tc.tile_pool(name="psc", bufs=2, space="PSUM"))
    stage = ctx.enter_context(tc.tile_pool(name="stage", bufs=16))

    ident = const.tile([128, 128], f32, name="ident", tag="ident")
    make_identity(nc, ident[:])

    # ---- input DMAs first (audio on the sync queue, block-granular) ----
    xin = [data.tile([128, NBLK, L], f32, name=f"xin{r}", tag=f"xin{r}")
           for r in range(2)]
    qi = 0
    for blk in range(NBLK):
        for r in range(2):
            asrc = audio[r].rearrange("(b p j) -> b p j", p=128, j=L)
            dq = nc.sync if qi % 2 == 0 else nc.scalar
            dq.dma_start(xin[r][:, blk, :], asrc[blk])
            qi += 1

    # ---- constants on the scalar HWDGE queue ----
    gt_sb = [const.tile([128, NQ, L], f32r, name=f"gt{r}", tag=f"gt{r}")
             for r in range(2)]
    ut_sb = [const.tile([128, NQ, NST], f32r, name=f"ut{r}", tag=f"ut{r}")
             for r in range(2)]
    kt_sb = [const.tile([NST, L], f32r, name=f"kt{r}", tag=f"kt{r}")
             for r in range(2)]
    up_sb = [[const.tile([128, 128], f32, name=f"up{r}_{v}", tag=f"up{r}_{v}")
              for v in range(N_LEV)] for r in range(2)]
    dn_sb = [[const.tile([NST, 128], f32, name=f"dn{r}_{v}", tag=f"dn{r}_{v}")
              for v in range(N_LEV)] for r in range(2)]
    for r in range(2):
        nc.sync.dma_start(ut_sb[r][:], ut[r].bitcast(f32r))
        nc.sync.dma_start(kt_sb[r][:], kt[r].bitcast(f32r))
        for v in range(N_LEV):
            nc.sync.dma_start(up_sb[r][v][:], sup[r, v])
            nc.sync.dma_start(dn_sb[r][v][:], sdn[r, v])
    for r in range(2):
        nc.sync.dma_start(gt_sb[r][:], gt[r].bitcast(f32r))

    # ---- transpose x into [j, c] layout (f32r rounding at the copy) ----
    xT = [data.tile([128, NQ, C], f32r, name=f"xT{r}", tag=f"xT{r}")
          for r in range(2)]
    for blk in range(NBLK):
        for r in range(2):
            for q in range(NQ):
                tp = pst.tile([128, 128], f32, name="tp", tag="pst")
                nc.tensor.transpose(tp[:], xin[r][:, blk, q * 128:(q + 1) * 128],
                                    ident[:])
                dst = xT[r][:, q, blk * 128:(blk + 1) * 128]
                if q % 2 == 0:
                    nc.vector.tensor_copy(dst, tp[:])
                else:
                    nc.scalar.copy(dst, tp[:])

    # ---- phase 1a: D = U @ x ----
    D_sb = [data.tile([NST, C], f32, name=f"D{r}", tag=f"D{r}") for r in range(2)]
    for r in range(2):
        dp = pst.tile([NST, C], f32, name="dp", tag="pst")
        for q in range(NQ):
            nc.tensor.matmul(dp[:], ut_sb[r][:, q, :],
                             xT[r][:, q, :],
                             start=(q == 0), stop=(q == NQ - 1))
        nc.vector.tensor_copy(D_sb[r][:], dp[:])

    # ---- phase 2 (scan) and phase 1b (y_zs) interleaved: one scan step,
    #      then one y-block, so the in-order PE stream never stalls long ----
    E = [D_sb[r][:] for r in range(2)]
    upalls = [[], []]
    Sin = [None, None]
    S_sb = [None, None]

    def up_step(r, lev):
        n_in = SCAN_SIZES[lev]
        n_g = n_in // 4
        rhs4 = data.tile([128, n_g], f32, name=f"rhs4_{r}_{lev}", tag=f"rhs4_{r}_{lev}")
        Eg = E[r].rearrange("p (g b) -> p g b", b=4)
        for b in range(4):
            eng = nc.vector if b % 2 == 0 else nc.gpsimd
            eng.tensor_copy(rhs4[b * NST:(b + 1) * NST, :], Eg[:, :, b])
        upo = pst.tile([128, n_g], f32, name="upo", tag="pst")
        nc.tensor.matmul(upo[:], up_sb[r][lev][:], rhs4[:], start=True, stop=True)
        # rows 0:32 = E_out, rows 32:128 = Sloc[b=1..3]
        upall = data.tile([128, n_g], f32, name=f"upall_{r}_{lev}", tag=f"upall_{r}_{lev}")
        if r == 0:
            nc.vector.tensor_copy(upall[:], upo[:])
        else:
            nc.scalar.copy(upall[:], upo[:])
        upalls[r].append(upall)
        E[r] = upall[:NST, :]

    def top_step(r):
        Sin_t = data.tile([NST, 2], f32, name=f"sintop_{r}", tag=f"sintop_{r}")
        nc.any.memset(Sin_t[:, 0:1], 0.0)
        nc.vector.tensor_copy(Sin_t[:, 1:2], E[r][:, 0:1])
        Sin[r] = Sin_t[:]

    def down_step(r, lev):
        n_g = SCAN_SIZES[lev] // 4
        prop = pst.tile([128, n_g], f32, name="prop", tag="pst")
        nc.tensor.matmul(prop[:], dn_sb[r][lev][:], Sin[r], start=True, stop=True)
        full = data.tile([128, n_g], f32, name=f"full_{r}_{lev}", tag=f"full_{r}_{lev}")
        nc.scalar.copy(full[:NST, :], prop[:NST, :])
        nc.vector.tensor_tensor(full[NST:2 * NST, :], prop[NST:2 * NST, :],
                                upalls[r][lev][NST:2 * NST, :],
                                op=mybir.AluOpType.add)
        nc.vector.tensor_tensor(full[2 * NST:, :], prop[2 * NST:, :],
                                upalls[r][lev][2 * NST:, :],
                                op=mybir.AluOpType.add)
        sdt = f32r if lev == 0 else f32
        Snx = data.tile([NST, SCAN_SIZES[lev]], sdt, name=f"snx_{r}_{lev}", tag=f"snx_{r}_{lev}")
        Sg = Snx[:].rearrange("p (g b) -> p g b", b=4)
        for b in range(4):
            eng = nc.vector if b % 2 == 0 else nc.gpsimd
            eng.tensor_copy(Sg[:, :, b], full[b * NST:(b + 1) * NST, :])
        Sin[r] = Snx[:]
        if lev == 0:
            S_sb[r] = Snx[:]

    ystages = {}

    def y_block(r, blk):
        cs = slice(blk * 128, (blk + 1) * 128)
        yp = psy.tile([128, L], f32, name="yp", tag="psy")
        for q in range(NQ):
            nc.tensor.matmul(yp[:], xT[r][:, q, cs], gt_sb[r][:, q, :],
                             start=(q == 0), stop=(q == NQ - 1))
        ystage = stage.tile([128, L], f32, name="ystage", tag="ystage")
        if blk % 2 == 0:
            nc.vector.tensor_copy(ystage[:], yp[:])
        else:
            nc.scalar.copy(ystage[:], yp[:])
        ystages[(r, blk)] = ystage

    scan_steps = []
    for lev in range(N_LEV):
        for r in range(2):
            scan_steps.append(lambda r=r, lev=lev: up_step(r, lev))
    for r in range(2):
        scan_steps.append(lambda r=r: top_step(r))
    for lev in range(N_LEV - 1, -1, -1):
        for r in range(2):
            scan_steps.append(lambda r=r, lev=lev: down_step(r, lev))

    y_list = [(r, blk) for blk in range(NBLK) for r in range(2)]
    yi = 0
    for si, step in enumerate(scan_steps):
        step()
        if yi < len(y_list):
            y_block(*y_list[yi]); yi += 1
    while yi < len(y_list):
        y_block(*y_list[yi]); yi += 1

    # ---- phase 3: corrections, add, DMA out ----
    for blk in range(NBLK):
        for r in range(2):
            cs = slice(blk * 128, (blk + 1) * 128)
            ydst = yout[r].rearrange("(b p l) -> b p l", p=128, l=L)
            cp = psc.tile([128, L], f32, name="cp", tag="psc")
            nc.tensor.matmul(cp[:], S_sb[r][:, cs], kt_sb[r][:],
                             start=True, stop=True)
            ystage = ystages[(r, blk)]
            nc.vector.tensor_tensor(ystage[:], ystage[:], cp[:],
                                    op=mybir.AluOpType.add)
            dq = nc.sync if blk % 2 == 0 else nc.scalar
            dq.dma_start(ydst[blk], ystage[:])


def build_program():
    from contextlib import ExitStack

    import concourse.mybir as mybir
    import concourse.tile as tile
    from concourse import bacc

    nc = bacc.Bacc("TRN2", target_bir_lowering=False, debug=False,
                   num_devices=N_CORES)
    f32 = mybir.dt.float32
    aps = dict(
        audio=nc.dram_tensor("audio", [2, T], f32, kind="ExternalInput").ap(),
        gt=nc.dram_tensor("gt", [2, 128, NQ, L], f32, kind="ExternalInput").ap(),
        ut=nc.dram_tensor("ut", [2, 128, NQ, NST], f32, kind="ExternalInput").ap(),
        kt=nc.dram_tensor("kt", [2, NST, L], f32, kind="ExternalInput").ap(),
        scan_up=nc.dram_tensor("scan_up", [2, N_LEV, 128, 128], f32,
                               kind="ExternalInput").ap(),
        scan_down=nc.dram_tensor("scan_down", [2, N_LEV, NST, 128], f32,
                                 kind="ExternalInput").ap(),
        y=nc.dram_tensor("y", [2, T], f32, kind="ExternalOutput").ap(),
    )
    with tile.TileContext(nc) as tc:
        with ExitStack() as ctx:
            _emit(ctx, tc, nc, aps)
    nc.compile()
    return nc


def _get_program():
    global _PROGRAM
    if _PROGRAM is None:
        _PROGRAM = build_program()
    return _PROGRAM


def make_in_maps(audio, params):
    audio = np.ascontiguousarray(np.asarray(audio, np.float32))
    mats = host_matrices(np.asarray(params, np.float64))
    in_maps = []
    for i in range(N_CORES):
        s = slice(2 * i, 2 * i + 2)
        in_maps.append({
            "audio": audio[s],
            "gt": mats["gt"][s],
            "ut": mats["ut"][s],
            "kt": mats["kt"][s],
            "scan_up": mats["scan_up"][s],
            "scan_down": mats["scan_down"][s],
        })
    return in_maps


def kernel(audio, params):
    from concourse.bass_utils import run_bass_kernel_spmd

    nc = _get_program()
    in_maps = make_in_maps(audio, params)
    res = run_bass_kernel_spmd(nc, in_maps, list(range(N_CORES)))
    return np.concatenate([res.results[i]["y"] for i in range(N_CORES)],
                          axis=0).astype(np.float32)
